# revision 26
# baseline (speedup 1.0000x reference)
"""GQA attention kernel for 8 Trainium2 NeuronCores.

Problem: B=2, S=2048, D=1024, 16 Q heads / 4 KV heads (GQA), causal,
y = softmax((x@wq+bq)(x@wk+bk)^T / 8, causal) @ (x@wv+bv) @ wo + bo

Sharding: core c -> (batch b = c//4, kv-group g = c%4). Each core computes
its batch's attention for 4 Q heads (= 1 KV head) and the partial output
projection through wo[g*256:(g+1)*256, :]. Host sums the 4 partials per
batch and adds bo.

Device kernel layout (v2):
  - x arrives pre-transposed from host as xT [D, S] bf16: no PE transposes.
  - kT [128, S]: rows 0-63 = k^T, rows 64-127 = duplicate (SBUF->SBUF DMA),
    so the PE can run in 64x128 row-tiled mode with two concurrent tiles
    (T0 = SBUF partitions 0-63, T8 = partitions 64-127).
  - qT per head-pair [128, S]: rows 0-63 = even head, 64-127 = odd head.
  - scores for the two heads of a pair run CONCURRENTLY on T0/T8.
  - AV contraction (128 keys) is split into two 64-key halves on T0/T8
    accumulating into separate PSUM banks (accA/accB); summed during the
    softmax-normalize step on DVE. No PE mode switches inside attention.
  - softmax denominator comes from a ones-column appended to v; 1/den via
    DVE reciprocal_approx_fast, partition-broadcast on gpsimd, applied on
    DVE. exp() width-trimmed on causal-diagonal tiles.
"""

import sys
from contextlib import ExitStack

import numpy as np
import ml_dtypes

if "/opt/trn_rl_repo" not in sys.path:
    sys.path.insert(0, "/opt/trn_rl_repo")

import concourse.bass as bass
import concourse.tile as tile
from concourse import bacc, mybir
from concourse.masks import make_identity

B, S, D = 2, 2048, 1024
H, KVH, HD = 16, 4, 64
GQ = H // KVH        # 4 q heads per core
DG = GQ * HD         # 256 q dims per core
P = 128
KC = D // P          # 8 contraction chunks over D
NKT = S // P         # 16 key tiles
NQB = S // 512       # 4 query blocks
N_CORES = 8

DT = mybir.dt.float32
DTB = mybir.dt.bfloat16
AF = mybir.ActivationFunctionType
BF16 = ml_dtypes.bfloat16

_CACHE = {}

import os
# bisect flags (HW debug)
ROW_TILED = os.environ.get("K_ROW_TILED", "1") == "1"
APPROX_RECIP = os.environ.get("K_APPROX_RECIP", "1") == "1"


def build_nc():
    nc = bacc.Bacc(
        "TRN2",
        target_bir_lowering=False,
        debug=False,
        enable_asserts=False,
        num_devices=N_CORES,
    )
    xtd = nc.dram_tensor("xtd", [D, S], DTB, kind="ExternalInput").ap()
    wqd = nc.dram_tensor("wqd", [D, DG], DTB, kind="ExternalInput").ap()
    wkvd = nc.dram_tensor("wkvd", [D, 2 * HD], DTB, kind="ExternalInput").ap()
    wod = nc.dram_tensor("wod", [DG, D], DTB, kind="ExternalInput").ap()
    bqd = nc.dram_tensor("bqd", [DG, 1], DT, kind="ExternalInput").ap()
    bkvd = nc.dram_tensor("bkvd", [2 * HD, 1], DT, kind="ExternalInput").ap()
    out_p = nc.dram_tensor("out_p", [S, D], DT, kind="ExternalOutput").ap()

    with tile.TileContext(nc) as tc, ExitStack() as ctx:
        consts = ctx.enter_context(tc.tile_pool(name="consts", bufs=1))
        etp = ctx.enter_context(tc.tile_pool(name="etp", bufs=6))
        vtp = ctx.enter_context(tc.tile_pool(name="vtp", bufs=2))
        dnp = ctx.enter_context(tc.tile_pool(name="dnp", bufs=3))
        rbp = ctx.enter_context(tc.tile_pool(name="rbp", bufs=3))
        ysb = ctx.enter_context(tc.tile_pool(name="ysb", bufs=3))
        psP = ctx.enter_context(tc.tile_pool(name="psP", bufs=2, space="PSUM"))
        psS = ctx.enter_context(tc.tile_pool(name="psS", bufs=2, space="PSUM"))
        psA = ctx.enter_context(tc.tile_pool(name="psA", bufs=4, space="PSUM"))

        # persistent SBUF
        xT = [consts.tile([P, S], DTB, tag=f"xT{kc}", name=f"xT{kc}") for kc in range(KC)]
        kT = consts.tile([P, S], DTB, tag="kT")
        qT = [consts.tile([P, S], DTB, tag=f"qT{pr}", name=f"qT{pr}") for pr in range(2)]
        qTlo = (None if ROW_TILED else
                [consts.tile([HD, S], DTB, tag=f"qTlo{pr}", name=f"qTlo{pr}")
                 for pr in range(2)])
        vA = consts.tile([P, NKT, HD + 1], DTB, tag="vA")
        oT = [consts.tile([P, S], DTB, tag=f"oT{c}", name=f"oT{c}") for c in range(2)]
        wq_sb = [consts.tile([P, DG], DTB, tag=f"wq{kc}", name=f"wq{kc}") for kc in range(KC)]
        wkv_sb = [consts.tile([P, 2 * HD], DTB, tag=f"wkv{kc}", name=f"wkv{kc}") for kc in range(KC)]
        wo_sb = [consts.tile([P, D], DTB, tag=f"wo{c}", name=f"wo{c}") for c in range(2)]
        bq_sb = consts.tile([P, 2], DT, tag="bq")
        # rows 0-63 = bk (aligned with k's PSUM rows), 64-127 = bv
        bkv_sb = consts.tile([P, 1], DT, tag="bkv")
        ident = consts.tile([HD, HD], DTB, tag="ident")
        ones_row = consts.tile([1, HD], DTB, tag="ones")
        nc.vector.memset(ones_row, 1.0)

        # ---- DMA queues: sync/gpsimd/scalar can issue DMA; scalar only
        # used for prologue loads (ACT is exp-bound in steady state) ----
        QS = [nc.sync, nc.gpsimd, nc.scalar]
        qi = 0

        def dq():
            nonlocal qi
            qi += 1
            return QS[qi % len(QS)]

        # identity for the small v transposes
        make_identity(nc, ident)

        for kc in range(KC):
            dq().dma_start(wq_sb[kc], wqd[kc * P:(kc + 1) * P, :])
            dq().dma_start(wkv_sb[kc], wkvd[kc * P:(kc + 1) * P, :])
        for c in range(2):
            dq().dma_start(wo_sb[c], wod[c * P:(c + 1) * P, :])
        for mc in range(2):
            dq().dma_start(bq_sb[:, mc:mc + 1], bqd[mc * P:(mc + 1) * P, :])
        dq().dma_start(bkv_sb, bkvd[:, :])
        # x^T tiles: first s-block separately so projections start early
        for kc in range(KC):
            dq().dma_start(xT[kc][:, 0:512], xtd[kc * P:(kc + 1) * P, 0:512])
        for kc in range(KC):
            dq().dma_start(xT[kc][:, 512:S], xtd[kc * P:(kc + 1) * P, 512:S])
        nc.vector.memset(vA[:, :, HD:HD + 1], 1.0)

        def proj(nb):
            sl = slice(nb * 512, (nb + 1) * 512)
            # k|v fused projection
            ps2 = psP.tile([P, 512], DT, tag="pp", name="ps2")
            for kc in range(KC):
                nc.tensor.matmul(ps2, wkv_sb[kc], xT[kc][:, sl],
                                 start=(kc == 0), stop=(kc == KC - 1))
            nc.vector.tensor_scalar_add(kT[0:HD, sl], ps2[0:HD, :], bkv_sb[0:HD, :])
            nc.vector.tensor_scalar_add(kT[HD:P, sl], ps2[0:HD, :], bkv_sb[0:HD, :])
            vt = vtp.tile([HD, 512], DTB, tag="vt", name="vt")
            nc.vector.tensor_scalar_add(
                vt, ps2[HD:2 * HD, :], bkv_sb[HD:P, :])
            for j in range(4):
                kt_i = nb * 4 + j
                vps = psP.tile([P, HD], DTB, tag="pp", name="vps")
                nc.tensor.transpose(vps, vt[:, j * P:(j + 1) * P], ident)
                nc.vector.tensor_copy(vA[:, kt_i, 0:HD], vps)
            # q projection, one matmul group per head pair
            for mc in range(2):
                ps = psP.tile([P, 512], DT, tag="pp", name="psq")
                for kc in range(KC):
                    nc.tensor.matmul(
                        ps, wq_sb[kc][:, mc * P:(mc + 1) * P], xT[kc][:, sl],
                        start=(kc == 0), stop=(kc == KC - 1))
                nc.vector.tensor_scalar(
                    out=qT[mc][:, sl], in0=ps,
                    scalar1=0.125, scalar2=bq_sb[:, mc:mc + 1],
                    op0=mybir.AluOpType.mult, op1=mybir.AluOpType.add)
                if not ROW_TILED:
                    nc.vector.tensor_copy(qTlo[mc][:, sl], qT[mc][HD:P, sl])

        def attn_pair(qb, pr):
            """Attention for head pair pr (local heads 2pr, 2pr+1), q block qb."""
            qsl0 = qb * 512
            nkt = 4 * (qb + 1)
            accA = [psA.tile([HD + 1, 512], DT, tag="acc", name="accA") for _ in range(2)]
            accB = (
                [psA.tile([HD + 1, 512], DT, tag="acc", name="accB") for _ in range(2)]
                if ROW_TILED else [None, None])
            for kt in range(nkt):
                di = kt - 4 * qb
                off = di * P if di > 0 else 0
                wv_ = 512 - off
                ktsl = slice(kt * P, (kt + 1) * P)
                qssl = slice(qsl0 + off, qsl0 + 512)
                s0 = psS.tile([P, 512], DT, tag="ss", name="s0")
                s1 = psS.tile([P, 512], DT, tag="ss", name="s1")
                nc.tensor.matmul(s0[:, off:512], kT[0:HD, ktsl], qT[pr][0:HD, qssl],
                                 start=True, stop=True)
                if ROW_TILED:
                    nc.tensor.matmul(s1[:, off:512], kT[HD:P, ktsl], qT[pr][HD:P, qssl],
                                     start=True, stop=True)
                else:
                    nc.tensor.matmul(s1[:, off:512], kT[0:HD, ktsl], qTlo[pr][:, qssl],
                                     start=True, stop=True)
                et0 = etp.tile([P, 512], DTB, tag="et", name="et0")
                et1 = etp.tile([P, 512], DTB, tag="et", name="et1")
                nc.scalar.activation(et0[:, off:512], s0[:, off:512], AF.Exp)
                nc.scalar.activation(et1[:, off:512], s1[:, off:512], AF.Exp)
                if di >= 0:
                    # causal: keep where (qb*512 + off + f) - p - kt*128 >= 0,
                    # i.e. q_abs >= key_abs; the AP starts at column `off` so
                    # fold it into base.
                    base = qb * 512 + off - kt * P
                    nc.gpsimd.affine_select(
                        out=et0[:, off:512], in_=et0[:, off:512],
                        pattern=[[1, wv_]],
                        compare_op=mybir.AluOpType.is_ge, fill=0.0,
                        base=base, channel_multiplier=-1)
                    nc.gpsimd.affine_select(
                        out=et1[:, off:512], in_=et1[:, off:512],
                        pattern=[[1, wv_]],
                        compare_op=mybir.AluOpType.is_ge, fill=0.0,
                        base=base, channel_multiplier=-1)
                st = (kt == 0)
                sp = (kt == nkt - 1)
                o5 = slice(off, 512)
                if ROW_TILED:
                    # split-K AV: T0 half (keys 0-63) and T8 half (keys
                    # 64-127) run concurrently; consecutive instructions
                    # alternate banks.
                    nc.tensor.matmul(accA[0][:, o5], vA[0:HD, kt, :], et0[0:HD, o5], start=st, stop=sp)
                    nc.tensor.matmul(accB[1][:, o5], vA[HD:P, kt, :], et1[HD:P, o5], start=st, stop=sp)
                    nc.tensor.matmul(accB[0][:, o5], vA[HD:P, kt, :], et0[HD:P, o5], start=st, stop=sp)
                    nc.tensor.matmul(accA[1][:, o5], vA[0:HD, kt, :], et1[0:HD, o5], start=st, stop=sp)
                else:
                    nc.tensor.matmul(accA[0][:, o5], vA[:, kt, :], et0[:, o5], start=st, stop=sp)
                    nc.tensor.matmul(accA[1][:, o5], vA[:, kt, :], et1[:, o5], start=st, stop=sp)
            qsl = slice(qsl0, qsl0 + 512)
            for hh in range(2):
                h = 2 * pr + hh
                c, r0 = h // 2, (h % 2) * HD
                dens = dnp.tile([HD + 1, 512], DT, tag="dens", name="dens")
                if ROW_TILED:
                    # only one PSUM operand per DVE op: stage accB in SBUF
                    obf = dnp.tile([HD + 1, 512], DT, tag="obf", name="obf")
                    nc.vector.tensor_copy(obf, accB[hh])
                    nc.vector.tensor_add(
                        dens[HD:HD + 1, :], accA[hh][HD:HD + 1, :], obf[HD:HD + 1, :])
                else:
                    nc.vector.tensor_copy(dens[HD:HD + 1, :], accA[hh][HD:HD + 1, :])
                rec = dnp.tile([1, 512], DT, tag="rec", name="rec")
                if APPROX_RECIP:
                    nc.vector.reciprocal_approx_fast(
                        out=rec, in_=dens[HD:HD + 1, :])
                else:
                    nc.vector.reciprocal(rec, dens[HD:HD + 1, :])
                recb = dnp.tile([1, 512], DTB, tag="recb", name="recb")
                nc.vector.tensor_copy(recb, rec)
                # broadcast 1/den across the 64 head dims via a K=1 matmul
                rbps = psS.tile([HD, 512], DT, tag="ss", name="rbps")
                nc.tensor.matmul(rbps, ones_row, recb, start=True, stop=True)
                recB = rbp.tile([P, 512], DTB, tag="recB", name="recB")
                nc.vector.tensor_copy(recB[r0:r0 + HD, :], rbps)
                if ROW_TILED:
                    nc.vector.tensor_add(
                        oT[c][r0:r0 + HD, qsl], accA[hh][0:HD, :], obf[0:HD, :])
                    nc.vector.tensor_mul(
                        oT[c][r0:r0 + HD, qsl], oT[c][r0:r0 + HD, qsl], recB[r0:r0 + HD, :])
                else:
                    nc.vector.tensor_mul(
                        oT[c][r0:r0 + HD, qsl], accA[hh][0:HD, :], recB[r0:r0 + HD, :])

        def wo_block(bl):
            for st in range(bl * 4, bl * 4 + 4):
                yt = ysb.tile([P, D], DT, tag="y", name="yt")
                for nb2 in range(2):
                    yps = psP.tile([P, 512], DT, tag="pp", name="yps")
                    for c in range(2):
                        nc.tensor.matmul(
                            yps, oT[c][:, st * P:(st + 1) * P],
                            wo_sb[c][:, nb2 * 512:(nb2 + 1) * 512],
                            start=(c == 0), stop=(c == 1))
                    nc.vector.tensor_copy(yt[:, nb2 * 512:(nb2 + 1) * 512], yps)
                QS[st % 2].dma_start(out_p[st * P:(st + 1) * P, :], yt)

        for nb in range(4):
            proj(nb)
            if nb >= 1:
                wo_block(nb - 1)
            attn_pair(nb, 0)
            attn_pair(nb, 1)
        wo_block(3)

    nc.compile()
    return nc


def prepare_in_maps(x, wq, bq, wk, bk, wv, bv, wo):
    x = np.asarray(x, dtype=np.float32)
    xb = [np.ascontiguousarray(x[b].T).astype(BF16) for b in range(B)]
    wqb = np.asarray(wq, dtype=np.float32).astype(BF16)
    wkb = np.asarray(wk, dtype=np.float32).astype(BF16)
    wvb = np.asarray(wv, dtype=np.float32).astype(BF16)
    wob = np.asarray(wo, dtype=np.float32).astype(BF16)
    bq = np.asarray(bq, dtype=np.float32)
    bk = np.asarray(bk, dtype=np.float32)
    bv = np.asarray(bv, dtype=np.float32)

    in_maps = []
    for c in range(N_CORES):
        b, g = c // 4, c % 4
        sq = slice(g * DG, (g + 1) * DG)
        sk = slice(g * HD, (g + 1) * HD)
        in_maps.append({
            "xtd": xb[b],
            "wqd": np.ascontiguousarray(wqb[:, sq]),
            "wkvd": np.ascontiguousarray(
                np.concatenate([wkb[:, sk], wvb[:, sk]], axis=1)),
            "wod": np.ascontiguousarray(wob[sq, :]),
            "bqd": np.ascontiguousarray((bq[sq] * 0.125).reshape(DG, 1)),
            "bkvd": np.ascontiguousarray(
                np.concatenate([bk[sk], bv[sk]]).reshape(2 * HD, 1)),
        })
    return in_maps


def kernel(x, mask, wq, bq, wk, bk, wv, bv, wo, bo):
    bo = np.asarray(bo, dtype=np.float32)
    in_maps = prepare_in_maps(x, wq, bq, wk, bk, wv, bv, wo)
    results = _run(in_maps)

    out = np.empty((B, S, D), dtype=np.float32)
    for b in range(B):
        acc = results[b * 4 + 0]["out_p"].astype(np.float64)
        for g in range(1, 4):
            acc += results[b * 4 + g]["out_p"]
        out[b] = (acc + bo).astype(np.float32)
    return out


def _get_runner():
    """Build (once) a jitted shard_map callable executing the compiled
    kernel on 8 cores. Adapted from concourse.bass2jax.run_bass_via_pjrt,
    minus output-buffer donation so the callable is re-invokable for
    timing."""
    if "runner" in _CACHE:
        return _CACHE["runner"]
    import jax
    from jax.experimental.shard_map import shard_map
    from jax.sharding import Mesh, PartitionSpec
    from concourse import bass2jax
    from concourse.bass2jax import _bass_exec_p, install_neuronx_cc_hook

    install_neuronx_cc_hook()
    nc = build_nc()
    partition_name = (
        nc.partition_id_tensor.name if nc.partition_id_tensor else None
    )

    in_names, out_names, out_avals, zero_outs = [], [], [], []
    for alloc in nc.m.functions[0].allocations:
        if not isinstance(alloc, mybir.MemoryLocationSet):
            continue
        name = alloc.memorylocations[0].name
        if alloc.kind == "ExternalInput":
            if name != partition_name:
                in_names.append(name)
        elif alloc.kind == "ExternalOutput":
            out_names.append(name)
            shape = tuple(alloc.tensor_shape)
            dtype = mybir.dt.np(alloc.dtype)
            out_avals.append(jax.core.ShapedArray(shape, dtype))
            zero_outs.append(np.zeros(shape, dtype))
    n_params = len(in_names)
    all_names = in_names + out_names
    if partition_name is not None:
        all_names = all_names + [partition_name]

    def _body(*args):
        operands = list(args)
        if partition_name is not None:
            operands.append(bass2jax.partition_id_tensor())
        outs = _bass_exec_p.bind(
            *operands,
            out_avals=tuple(out_avals),
            in_names=tuple(all_names),
            out_names=tuple(out_names),
            lowering_input_output_aliases=(),
            sim_require_finite=True,
            sim_require_nnan=True,
            nc=nc,
        )
        return tuple(outs)

    devices = jax.devices()[:N_CORES]
    mesh = Mesh(np.asarray(devices), ("core",))
    n_all = n_params + len(out_names)
    sharded = jax.jit(
        shard_map(
            _body,
            mesh=mesh,
            in_specs=(PartitionSpec("core"),) * n_all,
            out_specs=(PartitionSpec("core"),) * len(out_names),
            check_rep=False,
        ),
        keep_unused=True,
    )
    runner = {
        "sharded": sharded,
        "in_names": in_names,
        "out_names": out_names,
        "out_avals": out_avals,
        "zero_outs": zero_outs,
        "mesh": mesh,
        "nc": nc,
    }
    _CACHE["runner"] = runner
    return runner


def _run(in_maps):
    r = _get_runner()
    concat_in = [
        np.concatenate([np.asarray(in_maps[c][n]) for c in range(N_CORES)], axis=0)
        for n in r["in_names"]
    ]
    concat_zeros = [
        np.zeros((N_CORES * z.shape[0], *z.shape[1:]), z.dtype)
        for z in r["zero_outs"]
    ]
    out_arrs = r["sharded"](*concat_in, *concat_zeros)
    _CACHE["last_args"] = (concat_in, concat_zeros)
    return [
        {
            n: np.asarray(out_arrs[i]).reshape(
                N_CORES, *r["out_avals"][i].shape
            )[c]
            for i, n in enumerate(r["out_names"])
        }
        for c in range(N_CORES)
    ]


def bench(iters=10):
    """Re-execute the last-run kernel with device-resident inputs and
    return per-call wall times (s). Outputs stay on device."""
    import time as _time
    import jax
    from jax.sharding import NamedSharding, PartitionSpec

    r = _CACHE["runner"]
    concat_in, concat_zeros = _CACHE["last_args"]
    sh = NamedSharding(r["mesh"], PartitionSpec("core"))
    dev_args = [jax.device_put(a, sh) for a in (*concat_in, *concat_zeros)]
    for a in dev_args:
        a.block_until_ready()
    times = []
    for _ in range(iters):
        t0 = _time.perf_counter()
        outs = r["sharded"](*dev_args)
        for o in outs:
            o.block_until_ready()
        times.append(_time.perf_counter() - t0)
    return times


# revision 28
# speedup vs baseline: 291.9310x; 291.9310x over previous
"""GQA attention kernel for 8 Trainium2 NeuronCores.

Problem: B=2, S=2048, D=1024, 16 Q heads / 4 KV heads (GQA), causal,
y = softmax((x@wq+bq)(x@wk+bk)^T / 8, causal) @ (x@wv+bv) @ wo + bo

Sharding: core c -> (batch b = c//4, kv-group g = c%4). Each core computes
its batch's attention for 4 Q heads (= 1 KV head) and the partial output
projection through wo[g*256:(g+1)*256, :]. Host sums the 4 partials per
batch and adds bo.

Device kernel layout (v2):
  - x arrives pre-transposed from host as xT [D, S] bf16: no PE transposes.
  - kT [128, S]: rows 0-63 = k^T, rows 64-127 = duplicate (SBUF->SBUF DMA),
    so the PE can run in 64x128 row-tiled mode with two concurrent tiles
    (T0 = SBUF partitions 0-63, T8 = partitions 64-127).
  - qT per head-pair [128, S]: rows 0-63 = even head, 64-127 = odd head.
  - scores for the two heads of a pair run CONCURRENTLY on T0/T8.
  - AV contraction (128 keys) is split into two 64-key halves on T0/T8
    accumulating into separate PSUM banks (accA/accB); summed during the
    softmax-normalize step on DVE. No PE mode switches inside attention.
  - softmax denominator comes from a ones-column appended to v; 1/den via
    DVE reciprocal_approx_fast, partition-broadcast on gpsimd, applied on
    DVE. exp() width-trimmed on causal-diagonal tiles.
"""

import sys
from contextlib import ExitStack

import numpy as np
import ml_dtypes

if "/opt/trn_rl_repo" not in sys.path:
    sys.path.insert(0, "/opt/trn_rl_repo")

import concourse.bass as bass
import concourse.tile as tile
from concourse import bacc, mybir
from concourse.masks import make_identity

B, S, D = 2, 2048, 1024
H, KVH, HD = 16, 4, 64
GQ = H // KVH        # 4 q heads per core
DG = GQ * HD         # 256 q dims per core
P = 128
KC = D // P          # 8 contraction chunks over D
NKT = S // P         # 16 key tiles
NQB = S // 512       # 4 query blocks
N_CORES = 8

DT = mybir.dt.float32
DTB = mybir.dt.bfloat16
AF = mybir.ActivationFunctionType
BF16 = ml_dtypes.bfloat16

_CACHE = {}

import os
# bisect flags (HW debug)
ROW_TILED = os.environ.get("K_ROW_TILED", "1") == "1"
APPROX_RECIP = os.environ.get("K_APPROX_RECIP", "1") == "1"


def build_nc():
    nc = bacc.Bacc(
        "TRN2",
        target_bir_lowering=False,
        debug=False,
        enable_asserts=False,
        num_devices=N_CORES,
    )
    xtd = nc.dram_tensor("xtd", [D, S], DTB, kind="ExternalInput").ap()
    wqd = nc.dram_tensor("wqd", [D, DG], DTB, kind="ExternalInput").ap()
    wkvd = nc.dram_tensor("wkvd", [D, 2 * HD], DTB, kind="ExternalInput").ap()
    wod = nc.dram_tensor("wod", [DG, D], DTB, kind="ExternalInput").ap()
    bqd = nc.dram_tensor("bqd", [DG, 1], DT, kind="ExternalInput").ap()
    bkvd = nc.dram_tensor("bkvd", [2 * HD, 1], DT, kind="ExternalInput").ap()
    out_p = nc.dram_tensor("out_p", [S, D], DT, kind="ExternalOutput").ap()

    with tile.TileContext(nc) as tc, ExitStack() as ctx:
        consts = ctx.enter_context(tc.tile_pool(name="consts", bufs=1))
        etp = ctx.enter_context(tc.tile_pool(name="etp", bufs=6))
        vtp = ctx.enter_context(tc.tile_pool(name="vtp", bufs=2))
        dnp = ctx.enter_context(tc.tile_pool(name="dnp", bufs=3))
        rbp = ctx.enter_context(tc.tile_pool(name="rbp", bufs=3))
        ysb = ctx.enter_context(tc.tile_pool(name="ysb", bufs=3))
        psP = ctx.enter_context(tc.tile_pool(name="psP", bufs=2, space="PSUM"))
        psS = ctx.enter_context(tc.tile_pool(name="psS", bufs=2, space="PSUM"))
        psA = ctx.enter_context(tc.tile_pool(name="psA", bufs=4, space="PSUM"))

        # persistent SBUF
        xT = [consts.tile([P, S], DTB, tag=f"xT{kc}", name=f"xT{kc}") for kc in range(KC)]
        kT = consts.tile([P, S], DTB, tag="kT")
        qT = [consts.tile([P, S], DTB, tag=f"qT{pr}", name=f"qT{pr}") for pr in range(2)]
        qTlo = (None if ROW_TILED else
                [consts.tile([HD, S], DTB, tag=f"qTlo{pr}", name=f"qTlo{pr}")
                 for pr in range(2)])
        vA = consts.tile([P, NKT, HD + 1], DTB, tag="vA")
        oT = [consts.tile([P, S], DTB, tag=f"oT{c}", name=f"oT{c}") for c in range(2)]
        wq_sb = [consts.tile([P, DG], DTB, tag=f"wq{kc}", name=f"wq{kc}") for kc in range(KC)]
        wkv_sb = [consts.tile([P, 2 * HD], DTB, tag=f"wkv{kc}", name=f"wkv{kc}") for kc in range(KC)]
        wo_sb = [consts.tile([P, D], DTB, tag=f"wo{c}", name=f"wo{c}") for c in range(2)]
        bq_sb = consts.tile([P, 2], DT, tag="bq")
        # rows 0-63 = bk (aligned with k's PSUM rows), 64-127 = bv
        bkv_sb = consts.tile([P, 1], DT, tag="bkv")
        ident = consts.tile([HD, HD], DTB, tag="ident")
        ones_row = consts.tile([1, HD], DTB, tag="ones")
        nc.vector.memset(ones_row, 1.0)

        # ---- DMA queues: sync/gpsimd/scalar can issue DMA; scalar only
        # used for prologue loads (ACT is exp-bound in steady state) ----
        QS = [nc.sync, nc.gpsimd, nc.scalar]
        qi = 0

        def dq():
            nonlocal qi
            qi += 1
            return QS[qi % len(QS)]

        # identity for the small v transposes
        make_identity(nc, ident)

        for kc in range(KC):
            dq().dma_start(wq_sb[kc], wqd[kc * P:(kc + 1) * P, :])
            dq().dma_start(wkv_sb[kc], wkvd[kc * P:(kc + 1) * P, :])
        for c in range(2):
            dq().dma_start(wo_sb[c], wod[c * P:(c + 1) * P, :])
        for mc in range(2):
            dq().dma_start(bq_sb[:, mc:mc + 1], bqd[mc * P:(mc + 1) * P, :])
        dq().dma_start(bkv_sb, bkvd[:, :])
        # x^T tiles: first s-block separately so projections start early
        for kc in range(KC):
            dq().dma_start(xT[kc][:, 0:512], xtd[kc * P:(kc + 1) * P, 0:512])
        for kc in range(KC):
            dq().dma_start(xT[kc][:, 512:S], xtd[kc * P:(kc + 1) * P, 512:S])
        nc.vector.memset(vA[:, :, HD:HD + 1], 1.0)

        def proj(nb):
            sl = slice(nb * 512, (nb + 1) * 512)
            # k|v fused projection
            ps2 = psP.tile([P, 512], DT, tag="pp", name="ps2")
            for kc in range(KC):
                nc.tensor.matmul(ps2, wkv_sb[kc], xT[kc][:, sl],
                                 start=(kc == 0), stop=(kc == KC - 1))
            nc.vector.tensor_scalar_add(kT[0:HD, sl], ps2[0:HD, :], bkv_sb[0:HD, :])
            nc.sync.dma_start(kT[HD:P, sl], kT[0:HD, sl])
            vt = vtp.tile([HD, 512], DTB, tag="vt", name="vt")
            nc.vector.tensor_scalar_add(
                vt, ps2[HD:2 * HD, :], bkv_sb[HD:P, :])
            for j in range(4):
                kt_i = nb * 4 + j
                vps = psP.tile([P, HD], DTB, tag="pp", name="vps")
                nc.tensor.transpose(vps, vt[:, j * P:(j + 1) * P], ident)
                nc.vector.tensor_copy(vA[:, kt_i, 0:HD], vps)
            # q projection, one matmul group per head pair
            for mc in range(2):
                ps = psP.tile([P, 512], DT, tag="pp", name="psq")
                for kc in range(KC):
                    nc.tensor.matmul(
                        ps, wq_sb[kc][:, mc * P:(mc + 1) * P], xT[kc][:, sl],
                        start=(kc == 0), stop=(kc == KC - 1))
                nc.vector.tensor_scalar(
                    out=qT[mc][:, sl], in0=ps,
                    scalar1=0.125, scalar2=bq_sb[:, mc:mc + 1],
                    op0=mybir.AluOpType.mult, op1=mybir.AluOpType.add)
                if not ROW_TILED:
                    nc.vector.tensor_copy(qTlo[mc][:, sl], qT[mc][HD:P, sl])

        def attn_pair(qb, pr):
            """Attention for head pair pr (local heads 2pr, 2pr+1), q block qb."""
            qsl0 = qb * 512
            nkt = 4 * (qb + 1)
            accA = [psA.tile([HD + 1, 512], DT, tag="acc", name="accA") for _ in range(2)]
            accB = (
                [psA.tile([HD + 1, 512], DT, tag="acc", name="accB") for _ in range(2)]
                if ROW_TILED else [None, None])
            for kt in range(nkt):
                di = kt - 4 * qb
                off = di * P if di > 0 else 0
                wv_ = 512 - off
                ktsl = slice(kt * P, (kt + 1) * P)
                qssl = slice(qsl0 + off, qsl0 + 512)
                s0 = psS.tile([P, 512], DT, tag="ss", name="s0")
                s1 = psS.tile([P, 512], DT, tag="ss", name="s1")
                nc.tensor.matmul(s0[:, off:512], kT[0:HD, ktsl], qT[pr][0:HD, qssl],
                                 start=True, stop=True)
                if ROW_TILED:
                    nc.tensor.matmul(s1[:, off:512], kT[HD:P, ktsl], qT[pr][HD:P, qssl],
                                     start=True, stop=True)
                else:
                    nc.tensor.matmul(s1[:, off:512], kT[0:HD, ktsl], qTlo[pr][:, qssl],
                                     start=True, stop=True)
                et0 = etp.tile([P, 512], DTB, tag="et", name="et0")
                et1 = etp.tile([P, 512], DTB, tag="et", name="et1")
                nc.scalar.activation(et0[:, off:512], s0[:, off:512], AF.Exp)
                nc.scalar.activation(et1[:, off:512], s1[:, off:512], AF.Exp)
                if di >= 0:
                    # causal: keep where (qb*512 + off + f) - p - kt*128 >= 0,
                    # i.e. q_abs >= key_abs; the AP starts at column `off` so
                    # fold it into base.
                    base = qb * 512 + off - kt * P
                    nc.gpsimd.affine_select(
                        out=et0[:, off:512], in_=et0[:, off:512],
                        pattern=[[1, wv_]],
                        compare_op=mybir.AluOpType.is_ge, fill=0.0,
                        base=base, channel_multiplier=-1)
                    nc.gpsimd.affine_select(
                        out=et1[:, off:512], in_=et1[:, off:512],
                        pattern=[[1, wv_]],
                        compare_op=mybir.AluOpType.is_ge, fill=0.0,
                        base=base, channel_multiplier=-1)
                st = (kt == 0)
                sp = (kt == nkt - 1)
                o5 = slice(off, 512)
                if ROW_TILED:
                    # split-K AV: T0 half (keys 0-63) and T8 half (keys
                    # 64-127) run concurrently; consecutive instructions
                    # alternate banks.
                    nc.tensor.matmul(accA[0][:, o5], vA[0:HD, kt, :], et0[0:HD, o5], start=st, stop=sp)
                    nc.tensor.matmul(accB[1][:, o5], vA[HD:P, kt, :], et1[HD:P, o5], start=st, stop=sp)
                    nc.tensor.matmul(accB[0][:, o5], vA[HD:P, kt, :], et0[HD:P, o5], start=st, stop=sp)
                    nc.tensor.matmul(accA[1][:, o5], vA[0:HD, kt, :], et1[0:HD, o5], start=st, stop=sp)
                else:
                    nc.tensor.matmul(accA[0][:, o5], vA[:, kt, :], et0[:, o5], start=st, stop=sp)
                    nc.tensor.matmul(accA[1][:, o5], vA[:, kt, :], et1[:, o5], start=st, stop=sp)
            qsl = slice(qsl0, qsl0 + 512)
            for hh in range(2):
                h = 2 * pr + hh
                c, r0 = h // 2, (h % 2) * HD
                dens = dnp.tile([HD + 1, 512], DT, tag="dens", name="dens")
                if ROW_TILED:
                    # only one PSUM operand per DVE op: stage accB in SBUF
                    obf = dnp.tile([HD + 1, 512], DT, tag="obf", name="obf")
                    nc.vector.tensor_copy(obf, accB[hh])
                    nc.vector.tensor_add(
                        dens[HD:HD + 1, :], accA[hh][HD:HD + 1, :], obf[HD:HD + 1, :])
                else:
                    nc.vector.tensor_copy(dens[HD:HD + 1, :], accA[hh][HD:HD + 1, :])
                recb = dnp.tile([1, 512], DTB, tag="recb", name="recb")
                if APPROX_RECIP:
                    # custom-DVE op: keep in/out partition-aligned (row 64)
                    rec = dnp.tile([HD + 1, 512], DT, tag="rec", name="rec")
                    nc.vector.reciprocal_approx_fast(
                        out=rec[HD:HD + 1, :], in_=dens[HD:HD + 1, :])
                    nc.vector.tensor_copy(recb, rec[HD:HD + 1, :])
                else:
                    rec = dnp.tile([1, 512], DT, tag="rec", name="rec")
                    nc.vector.reciprocal(rec, dens[HD:HD + 1, :])
                    nc.vector.tensor_copy(recb, rec)
                # broadcast 1/den across the 64 head dims via a K=1 matmul
                rbps = psS.tile([HD, 512], DT, tag="ss", name="rbps")
                nc.tensor.matmul(rbps, ones_row, recb, start=True, stop=True)
                recB = rbp.tile([P, 512], DTB, tag="recB", name="recB")
                nc.vector.tensor_copy(recB[r0:r0 + HD, :], rbps)
                if ROW_TILED:
                    nc.vector.tensor_add(
                        oT[c][r0:r0 + HD, qsl], accA[hh][0:HD, :], obf[0:HD, :])
                    nc.vector.tensor_mul(
                        oT[c][r0:r0 + HD, qsl], oT[c][r0:r0 + HD, qsl], recB[r0:r0 + HD, :])
                else:
                    nc.vector.tensor_mul(
                        oT[c][r0:r0 + HD, qsl], accA[hh][0:HD, :], recB[r0:r0 + HD, :])

        def wo_block(bl):
            for st in range(bl * 4, bl * 4 + 4):
                yt = ysb.tile([P, D], DT, tag="y", name="yt")
                for nb2 in range(2):
                    yps = psP.tile([P, 512], DT, tag="pp", name="yps")
                    for c in range(2):
                        nc.tensor.matmul(
                            yps, oT[c][:, st * P:(st + 1) * P],
                            wo_sb[c][:, nb2 * 512:(nb2 + 1) * 512],
                            start=(c == 0), stop=(c == 1))
                    nc.vector.tensor_copy(yt[:, nb2 * 512:(nb2 + 1) * 512], yps)
                QS[st % 2].dma_start(out_p[st * P:(st + 1) * P, :], yt)

        for nb in range(4):
            proj(nb)
            if nb >= 1:
                wo_block(nb - 1)
            attn_pair(nb, 0)
            attn_pair(nb, 1)
        wo_block(3)

    nc.compile()
    return nc


def prepare_in_maps(x, wq, bq, wk, bk, wv, bv, wo):
    x = np.asarray(x, dtype=np.float32)
    xb = [np.ascontiguousarray(x[b].T).astype(BF16) for b in range(B)]
    wqb = np.asarray(wq, dtype=np.float32).astype(BF16)
    wkb = np.asarray(wk, dtype=np.float32).astype(BF16)
    wvb = np.asarray(wv, dtype=np.float32).astype(BF16)
    wob = np.asarray(wo, dtype=np.float32).astype(BF16)
    bq = np.asarray(bq, dtype=np.float32)
    bk = np.asarray(bk, dtype=np.float32)
    bv = np.asarray(bv, dtype=np.float32)

    in_maps = []
    for c in range(N_CORES):
        b, g = c // 4, c % 4
        sq = slice(g * DG, (g + 1) * DG)
        sk = slice(g * HD, (g + 1) * HD)
        in_maps.append({
            "xtd": xb[b],
            "wqd": np.ascontiguousarray(wqb[:, sq]),
            "wkvd": np.ascontiguousarray(
                np.concatenate([wkb[:, sk], wvb[:, sk]], axis=1)),
            "wod": np.ascontiguousarray(wob[sq, :]),
            "bqd": np.ascontiguousarray((bq[sq] * 0.125).reshape(DG, 1)),
            "bkvd": np.ascontiguousarray(
                np.concatenate([bk[sk], bv[sk]]).reshape(2 * HD, 1)),
        })
    return in_maps


def kernel(x, mask, wq, bq, wk, bk, wv, bv, wo, bo):
    bo = np.asarray(bo, dtype=np.float32)
    in_maps = prepare_in_maps(x, wq, bq, wk, bk, wv, bv, wo)
    results = _run(in_maps)

    out = np.empty((B, S, D), dtype=np.float32)
    for b in range(B):
        acc = results[b * 4 + 0]["out_p"].astype(np.float64)
        for g in range(1, 4):
            acc += results[b * 4 + g]["out_p"]
        out[b] = (acc + bo).astype(np.float32)
    return out


def _get_runner():
    """Build (once) a jitted shard_map callable executing the compiled
    kernel on 8 cores. Adapted from concourse.bass2jax.run_bass_via_pjrt,
    minus output-buffer donation so the callable is re-invokable for
    timing."""
    if "runner" in _CACHE:
        return _CACHE["runner"]
    import jax
    from jax.experimental.shard_map import shard_map
    from jax.sharding import Mesh, PartitionSpec
    from concourse import bass2jax
    from concourse.bass2jax import _bass_exec_p, install_neuronx_cc_hook

    install_neuronx_cc_hook()
    nc = build_nc()
    partition_name = (
        nc.partition_id_tensor.name if nc.partition_id_tensor else None
    )

    in_names, out_names, out_avals, zero_outs = [], [], [], []
    for alloc in nc.m.functions[0].allocations:
        if not isinstance(alloc, mybir.MemoryLocationSet):
            continue
        name = alloc.memorylocations[0].name
        if alloc.kind == "ExternalInput":
            if name != partition_name:
                in_names.append(name)
        elif alloc.kind == "ExternalOutput":
            out_names.append(name)
            shape = tuple(alloc.tensor_shape)
            dtype = mybir.dt.np(alloc.dtype)
            out_avals.append(jax.core.ShapedArray(shape, dtype))
            zero_outs.append(np.zeros(shape, dtype))
    n_params = len(in_names)
    all_names = in_names + out_names
    if partition_name is not None:
        all_names = all_names + [partition_name]

    def _body(*args):
        operands = list(args)
        if partition_name is not None:
            operands.append(bass2jax.partition_id_tensor())
        outs = _bass_exec_p.bind(
            *operands,
            out_avals=tuple(out_avals),
            in_names=tuple(all_names),
            out_names=tuple(out_names),
            lowering_input_output_aliases=(),
            sim_require_finite=True,
            sim_require_nnan=True,
            nc=nc,
        )
        return tuple(outs)

    devices = jax.devices()[:N_CORES]
    mesh = Mesh(np.asarray(devices), ("core",))
    n_all = n_params + len(out_names)
    sharded = jax.jit(
        shard_map(
            _body,
            mesh=mesh,
            in_specs=(PartitionSpec("core"),) * n_all,
            out_specs=(PartitionSpec("core"),) * len(out_names),
            check_rep=False,
        ),
        keep_unused=True,
    )
    runner = {
        "sharded": sharded,
        "in_names": in_names,
        "out_names": out_names,
        "out_avals": out_avals,
        "zero_outs": zero_outs,
        "mesh": mesh,
        "nc": nc,
    }
    _CACHE["runner"] = runner
    return runner


def _run(in_maps):
    r = _get_runner()
    concat_in = [
        np.concatenate([np.asarray(in_maps[c][n]) for c in range(N_CORES)], axis=0)
        for n in r["in_names"]
    ]
    concat_zeros = [
        np.zeros((N_CORES * z.shape[0], *z.shape[1:]), z.dtype)
        for z in r["zero_outs"]
    ]
    out_arrs = r["sharded"](*concat_in, *concat_zeros)
    _CACHE["last_args"] = (concat_in, concat_zeros)
    return [
        {
            n: np.asarray(out_arrs[i]).reshape(
                N_CORES, *r["out_avals"][i].shape
            )[c]
            for i, n in enumerate(r["out_names"])
        }
        for c in range(N_CORES)
    ]


def bench(iters=10):
    """Re-execute the last-run kernel with device-resident inputs and
    return per-call wall times (s). Outputs stay on device."""
    import time as _time
    import jax
    from jax.sharding import NamedSharding, PartitionSpec

    r = _CACHE["runner"]
    concat_in, concat_zeros = _CACHE["last_args"]
    sh = NamedSharding(r["mesh"], PartitionSpec("core"))
    dev_args = [jax.device_put(a, sh) for a in (*concat_in, *concat_zeros)]
    for a in dev_args:
        a.block_until_ready()
    times = []
    for _ in range(iters):
        t0 = _time.perf_counter()
        outs = r["sharded"](*dev_args)
        for o in outs:
            o.block_until_ready()
        times.append(_time.perf_counter() - t0)
    return times


# revision 35
# speedup vs baseline: 320.5756x; 1.0981x over previous
"""GQA attention kernel for 8 Trainium2 NeuronCores.

Problem: B=2, S=2048, D=1024, 16 Q heads / 4 KV heads (GQA), causal,
y = softmax((x@wq+bq)(x@wk+bk)^T / 8, causal) @ (x@wv+bv) @ wo + bo

Sharding: core c -> (batch b = c//4, kv-group g = c%4). Each core computes
its batch's attention for 4 Q heads (= 1 KV head) and the partial output
projection through wo[g*256:(g+1)*256, :]. Host sums the 4 partials per
batch and adds bo.

Device kernel layout (v2):
  - x arrives pre-transposed from host as xT [D, S] bf16: no PE transposes.
  - kT [128, S]: rows 0-63 = k^T, rows 64-127 = duplicate (SBUF->SBUF DMA),
    so the PE can run in 64x128 row-tiled mode with two concurrent tiles
    (T0 = SBUF partitions 0-63, T8 = partitions 64-127).
  - qT per head-pair [128, S]: rows 0-63 = even head, 64-127 = odd head.
  - scores for the two heads of a pair run CONCURRENTLY on T0/T8.
  - AV contraction (128 keys) is split into two 64-key halves on T0/T8
    accumulating into separate PSUM banks (accA/accB); summed during the
    softmax-normalize step on DVE. No PE mode switches inside attention.
  - softmax denominator comes from a ones-column appended to v; 1/den via
    DVE reciprocal_approx_fast, partition-broadcast on gpsimd, applied on
    DVE. exp() width-trimmed on causal-diagonal tiles.
"""

import sys
from contextlib import ExitStack

import numpy as np
import ml_dtypes

if "/opt/trn_rl_repo" not in sys.path:
    sys.path.insert(0, "/opt/trn_rl_repo")

import concourse.bass as bass
import concourse.tile as tile
from concourse import bacc, mybir
from concourse.masks import make_identity

B, S, D = 2, 2048, 1024
H, KVH, HD = 16, 4, 64
GQ = H // KVH        # 4 q heads per core
DG = GQ * HD         # 256 q dims per core
P = 128
KC = D // P          # 8 contraction chunks over D
NKT = S // P         # 16 key tiles
NQB = S // 512       # 4 query blocks
N_CORES = 8

DT = mybir.dt.float32
DTB = mybir.dt.bfloat16
AF = mybir.ActivationFunctionType
BF16 = ml_dtypes.bfloat16

_CACHE = {}

import os
# bisect flags (HW debug)
ROW_TILED = os.environ.get("K_ROW_TILED", "1") == "1"
# reciprocal_approx_fast (custom DVE ucode) and gpsimd partition_broadcast
# both produce garbage on HW through this execution path (sim is fine) —
# keep them off; plain DVE reciprocal + ones-row matmul broadcast work.
APPROX_RECIP = os.environ.get("K_APPROX_RECIP", "0") == "1"
PBCAST = os.environ.get("K_PBCAST", "0") == "1"
FILL = os.environ.get("K_FILL", "1") == "1"


def build_nc():
    nc = bacc.Bacc(
        "TRN2",
        target_bir_lowering=False,
        debug=False,
        enable_asserts=False,
        num_devices=N_CORES,
    )
    xtd = nc.dram_tensor("xtd", [D, S], DTB, kind="ExternalInput").ap()
    wqd = nc.dram_tensor("wqd", [D, DG], DTB, kind="ExternalInput").ap()
    wkvd = nc.dram_tensor("wkvd", [D, 2 * HD], DTB, kind="ExternalInput").ap()
    wod = nc.dram_tensor("wod", [DG, D], DTB, kind="ExternalInput").ap()
    bqd = nc.dram_tensor("bqd", [DG, 1], DT, kind="ExternalInput").ap()
    bkvd = nc.dram_tensor("bkvd", [2 * HD, 1], DT, kind="ExternalInput").ap()
    out_p = nc.dram_tensor("out_p", [S, D], DT, kind="ExternalOutput").ap()

    with tile.TileContext(nc) as tc, ExitStack() as ctx:
        consts = ctx.enter_context(tc.tile_pool(name="consts", bufs=1))
        etp = ctx.enter_context(tc.tile_pool(name="etp", bufs=6))
        vtp = ctx.enter_context(tc.tile_pool(name="vtp", bufs=2))
        dnp = ctx.enter_context(tc.tile_pool(name="dnp", bufs=3))
        rbp = ctx.enter_context(tc.tile_pool(name="rbp", bufs=3))
        ysb = ctx.enter_context(tc.tile_pool(name="ysb", bufs=3))
        psP = ctx.enter_context(tc.tile_pool(name="psP", bufs=2, space="PSUM"))
        psS = ctx.enter_context(tc.tile_pool(name="psS", bufs=2, space="PSUM"))
        psA = ctx.enter_context(tc.tile_pool(name="psA", bufs=4, space="PSUM"))

        # persistent SBUF
        xT = [consts.tile([P, S], DTB, tag=f"xT{kc}", name=f"xT{kc}") for kc in range(KC)]
        kT = consts.tile([P, S], DTB, tag="kT")
        qT = [consts.tile([P, S], DTB, tag=f"qT{pr}", name=f"qT{pr}") for pr in range(2)]
        qTlo = (None if ROW_TILED else
                [consts.tile([HD, S], DTB, tag=f"qTlo{pr}", name=f"qTlo{pr}")
                 for pr in range(2)])
        vA = consts.tile([P, NKT, HD + 1], DTB, tag="vA")
        oT = [consts.tile([P, S], DTB, tag=f"oT{c}", name=f"oT{c}") for c in range(2)]
        wq_sb = [consts.tile([P, DG], DTB, tag=f"wq{kc}", name=f"wq{kc}") for kc in range(KC)]
        wkv_sb = [consts.tile([P, 2 * HD], DTB, tag=f"wkv{kc}", name=f"wkv{kc}") for kc in range(KC)]
        wo_sb = [consts.tile([P, D], DTB, tag=f"wo{c}", name=f"wo{c}") for c in range(2)]
        bq_sb = consts.tile([P, 2], DT, tag="bq")
        # rows 0-63 = bk (aligned with k's PSUM rows), 64-127 = bv
        bkv_sb = consts.tile([P, 1], DT, tag="bkv")
        ident = consts.tile([HD, HD], DTB, tag="ident")
        ones_row = consts.tile([1, HD], DTB, tag="ones")
        nc.vector.memset(ones_row, 1.0)

        # ---- DMA queues: sync/gpsimd/scalar can issue DMA; scalar only
        # used for prologue loads (ACT is exp-bound in steady state) ----
        QS = [nc.sync, nc.gpsimd, nc.scalar]
        qi = 0

        def dq():
            nonlocal qi
            qi += 1
            return QS[qi % len(QS)]

        # identity for the small v transposes
        make_identity(nc, ident)

        for kc in range(KC):
            dq().dma_start(wq_sb[kc], wqd[kc * P:(kc + 1) * P, :])
            dq().dma_start(wkv_sb[kc], wkvd[kc * P:(kc + 1) * P, :])
        for c in range(2):
            dq().dma_start(wo_sb[c], wod[c * P:(c + 1) * P, :])
        for mc in range(2):
            dq().dma_start(bq_sb[:, mc:mc + 1], bqd[mc * P:(mc + 1) * P, :])
        dq().dma_start(bkv_sb, bkvd[:, :])
        # x^T tiles: first s-block separately so projections start early
        for kc in range(KC):
            dq().dma_start(xT[kc][:, 0:512], xtd[kc * P:(kc + 1) * P, 0:512])
        for kc in range(KC):
            dq().dma_start(xT[kc][:, 512:S], xtd[kc * P:(kc + 1) * P, 512:S])
        nc.vector.memset(vA[:, :, HD:HD + 1], 1.0)

        def proj_kv(nb):
            sl = slice(nb * 512, (nb + 1) * 512)
            # k|v fused projection
            ps2 = psP.tile([P, 512], DT, tag="pp", name="ps2")
            for kc in range(KC):
                nc.tensor.matmul(ps2, wkv_sb[kc], xT[kc][:, sl],
                                 start=(kc == 0), stop=(kc == KC - 1))
            nc.vector.tensor_scalar_add(kT[0:HD, sl], ps2[0:HD, :], bkv_sb[0:HD, :])
            nc.sync.dma_start(kT[HD:P, sl], kT[0:HD, sl])
            vt = vtp.tile([HD, 512], DTB, tag="vt", name="vt")
            nc.vector.tensor_scalar_add(
                vt, ps2[HD:2 * HD, :], bkv_sb[HD:P, :])
            return vt

        def proj_vtrans(nb, vt):
            for j in range(4):
                kt_i = nb * 4 + j
                vps = psP.tile([P, HD], DTB, tag="pp", name="vps")
                nc.tensor.transpose(vps, vt[:, j * P:(j + 1) * P], ident)
                nc.vector.tensor_copy(vA[:, kt_i, 0:HD], vps)

        def proj_q(nb, mc):
            sl = slice(nb * 512, (nb + 1) * 512)
            ps = psP.tile([P, 512], DT, tag="pp", name="psq")
            for kc in range(KC):
                nc.tensor.matmul(
                    ps, wq_sb[kc][:, mc * P:(mc + 1) * P], xT[kc][:, sl],
                    start=(kc == 0), stop=(kc == KC - 1))
            nc.vector.tensor_scalar(
                out=qT[mc][:, sl], in0=ps,
                scalar1=0.125, scalar2=bq_sb[:, mc:mc + 1],
                op0=mybir.AluOpType.mult, op1=mybir.AluOpType.add)
            if not ROW_TILED:
                nc.vector.tensor_copy(qTlo[mc][:, sl], qT[mc][HD:P, sl])

        def proj_pieces(nb):
            state = {}

            def p0():
                state["vt"] = proj_kv(nb)

            return [p0,
                    lambda: proj_vtrans(nb, state["vt"]),
                    lambda: proj_q(nb, 0),
                    lambda: proj_q(nb, 1)]

        def attn_pair(qb, pr, fill_iter=None):
            """Attention for head pair pr (local heads 2pr, 2pr+1), q block qb."""
            qsl0 = qb * 512
            nkt = 4 * (qb + 1)
            accA = [psA.tile([HD + 1, 512], DT, tag="acc", name="accA") for _ in range(2)]
            accB = (
                [psA.tile([HD + 1, 512], DT, tag="acc", name="accB") for _ in range(2)]
                if ROW_TILED else [None, None])
            for kt in range(nkt):
                di = kt - 4 * qb
                off = di * P if di > 0 else 0
                wv_ = 512 - off
                ktsl = slice(kt * P, (kt + 1) * P)
                qssl = slice(qsl0 + off, qsl0 + 512)
                s0 = psS.tile([P, 512], DT, tag="ss", name="s0")
                s1 = psS.tile([P, 512], DT, tag="ss", name="s1")
                nc.tensor.matmul(s0[:, off:512], kT[0:HD, ktsl], qT[pr][0:HD, qssl],
                                 start=True, stop=True)
                if ROW_TILED:
                    nc.tensor.matmul(s1[:, off:512], kT[HD:P, ktsl], qT[pr][HD:P, qssl],
                                     start=True, stop=True)
                else:
                    nc.tensor.matmul(s1[:, off:512], kT[0:HD, ktsl], qTlo[pr][:, qssl],
                                     start=True, stop=True)
                et0 = etp.tile([P, 512], DTB, tag="et", name="et0")
                et1 = etp.tile([P, 512], DTB, tag="et", name="et1")
                nc.scalar.activation(et0[:, off:512], s0[:, off:512], AF.Exp)
                nc.scalar.activation(et1[:, off:512], s1[:, off:512], AF.Exp)
                if di >= 0:
                    # causal: keep where (qb*512 + off + f) - p - kt*128 >= 0,
                    # i.e. q_abs >= key_abs; the AP starts at column `off` so
                    # fold it into base.
                    base = qb * 512 + off - kt * P
                    nc.gpsimd.affine_select(
                        out=et0[:, off:512], in_=et0[:, off:512],
                        pattern=[[1, wv_]],
                        compare_op=mybir.AluOpType.is_ge, fill=0.0,
                        base=base, channel_multiplier=-1)
                    nc.gpsimd.affine_select(
                        out=et1[:, off:512], in_=et1[:, off:512],
                        pattern=[[1, wv_]],
                        compare_op=mybir.AluOpType.is_ge, fill=0.0,
                        base=base, channel_multiplier=-1)
                if fill_iter is not None and kt % 2 == 1:
                    # interleave a projection/output-proj piece: the PE chews
                    # on it while ACT runs the exps, instead of idling.
                    f = next(fill_iter, None)
                    if f is not None:
                        f()
                st = (kt == 0)
                sp = (kt == nkt - 1)
                o5 = slice(off, 512)
                if ROW_TILED:
                    # split-K AV: T0 half (keys 0-63) and T8 half (keys
                    # 64-127) run concurrently; consecutive instructions
                    # alternate banks.
                    nc.tensor.matmul(accA[0][:, o5], vA[0:HD, kt, :], et0[0:HD, o5], start=st, stop=sp)
                    nc.tensor.matmul(accB[1][:, o5], vA[HD:P, kt, :], et1[HD:P, o5], start=st, stop=sp)
                    nc.tensor.matmul(accB[0][:, o5], vA[HD:P, kt, :], et0[HD:P, o5], start=st, stop=sp)
                    nc.tensor.matmul(accA[1][:, o5], vA[0:HD, kt, :], et1[0:HD, o5], start=st, stop=sp)
                else:
                    nc.tensor.matmul(accA[0][:, o5], vA[:, kt, :], et0[:, o5], start=st, stop=sp)
                    nc.tensor.matmul(accA[1][:, o5], vA[:, kt, :], et1[:, o5], start=st, stop=sp)
            qsl = slice(qsl0, qsl0 + 512)
            for hh in range(2):
                h = 2 * pr + hh
                c, r0 = h // 2, (h % 2) * HD
                dens = dnp.tile([HD + 1, 512], DT, tag="dens", name="dens")
                if ROW_TILED:
                    # only one PSUM operand per DVE op: stage accB in SBUF
                    obf = dnp.tile([HD + 1, 512], DT, tag="obf", name="obf")
                    nc.vector.tensor_copy(obf, accB[hh])
                    nc.vector.tensor_add(
                        dens[HD:HD + 1, :], accA[hh][HD:HD + 1, :], obf[HD:HD + 1, :])
                else:
                    nc.vector.tensor_copy(dens[HD:HD + 1, :], accA[hh][HD:HD + 1, :])
                recb = dnp.tile([1, 512], DTB, tag="recb", name="recb")
                if APPROX_RECIP:
                    # custom-DVE op: keep in/out partition-aligned (row 64)
                    rec = dnp.tile([HD + 1, 512], DT, tag="rec", name="rec")
                    nc.vector.reciprocal_approx_fast(
                        out=rec[HD:HD + 1, :], in_=dens[HD:HD + 1, :])
                    nc.vector.tensor_copy(recb, rec[HD:HD + 1, :])
                else:
                    rec = dnp.tile([1, 512], DT, tag="rec", name="rec")
                    nc.vector.reciprocal(rec, dens[HD:HD + 1, :])
                    nc.vector.tensor_copy(recb, rec)
                recB = rbp.tile([P, 512], DTB, tag="recB", name="recB")
                if PBCAST:
                    nc.gpsimd.partition_broadcast(
                        recB[r0:r0 + HD, :], recb, channels=HD)
                else:
                    # broadcast 1/den across 64 head dims via a K=1 matmul;
                    # psP (not psS) so the next pair's scores aren't blocked
                    rbps = psP.tile([HD, 512], DT, tag="pp", name="rbps")
                    nc.tensor.matmul(rbps, ones_row, recb, start=True, stop=True)
                    nc.vector.tensor_copy(recB[r0:r0 + HD, :], rbps)
                if ROW_TILED:
                    nc.vector.tensor_add(
                        oT[c][r0:r0 + HD, qsl], accA[hh][0:HD, :], obf[0:HD, :])
                    nc.vector.tensor_mul(
                        oT[c][r0:r0 + HD, qsl], oT[c][r0:r0 + HD, qsl], recB[r0:r0 + HD, :])
                else:
                    nc.vector.tensor_mul(
                        oT[c][r0:r0 + HD, qsl], accA[hh][0:HD, :], recB[r0:r0 + HD, :])

        def wo_st(st):
            yt = ysb.tile([P, D], DT, tag="y", name="yt")
            for nb2 in range(2):
                yps = psP.tile([P, 512], DT, tag="pp", name="yps")
                for c in range(2):
                    nc.tensor.matmul(
                        yps, oT[c][:, st * P:(st + 1) * P],
                        wo_sb[c][:, nb2 * 512:(nb2 + 1) * 512],
                        start=(c == 0), stop=(c == 1))
                nc.vector.tensor_copy(yt[:, nb2 * 512:(nb2 + 1) * 512], yps)
            QS[st % 2].dma_start(out_p[st * P:(st + 1) * P, :], yt)

        def wo_pieces(bl):
            return [(lambda st=st: wo_st(st)) for st in range(bl * 4, bl * 4 + 4)]

        if FILL:
            for f in proj_pieces(0):
                f()
            for qb in range(4):
                fillers = []
                if qb + 1 < 4:
                    fillers += proj_pieces(qb + 1)
                if qb >= 1:
                    fillers += wo_pieces(qb - 1)
                fill_iter = iter(fillers)
                attn_pair(qb, 0, fill_iter)
                attn_pair(qb, 1, fill_iter)
                for f in fill_iter:
                    f()
            for f in wo_pieces(3):
                f()
        else:
            for nb in range(4):
                for f in proj_pieces(nb):
                    f()
                if nb >= 1:
                    for f in wo_pieces(nb - 1):
                        f()
                attn_pair(nb, 0)
                attn_pair(nb, 1)
            for f in wo_pieces(3):
                f()

    nc.compile()
    return nc


def prepare_in_maps(x, wq, bq, wk, bk, wv, bv, wo):
    x = np.asarray(x, dtype=np.float32)
    xb = [np.ascontiguousarray(x[b].T).astype(BF16) for b in range(B)]
    wqb = np.asarray(wq, dtype=np.float32).astype(BF16)
    wkb = np.asarray(wk, dtype=np.float32).astype(BF16)
    wvb = np.asarray(wv, dtype=np.float32).astype(BF16)
    wob = np.asarray(wo, dtype=np.float32).astype(BF16)
    bq = np.asarray(bq, dtype=np.float32)
    bk = np.asarray(bk, dtype=np.float32)
    bv = np.asarray(bv, dtype=np.float32)

    in_maps = []
    for c in range(N_CORES):
        b, g = c // 4, c % 4
        sq = slice(g * DG, (g + 1) * DG)
        sk = slice(g * HD, (g + 1) * HD)
        in_maps.append({
            "xtd": xb[b],
            "wqd": np.ascontiguousarray(wqb[:, sq]),
            "wkvd": np.ascontiguousarray(
                np.concatenate([wkb[:, sk], wvb[:, sk]], axis=1)),
            "wod": np.ascontiguousarray(wob[sq, :]),
            "bqd": np.ascontiguousarray((bq[sq] * 0.125).reshape(DG, 1)),
            "bkvd": np.ascontiguousarray(
                np.concatenate([bk[sk], bv[sk]]).reshape(2 * HD, 1)),
        })
    return in_maps


def kernel(x, mask, wq, bq, wk, bk, wv, bv, wo, bo):
    bo = np.asarray(bo, dtype=np.float32)
    in_maps = prepare_in_maps(x, wq, bq, wk, bk, wv, bv, wo)
    results = _run(in_maps)

    out = np.empty((B, S, D), dtype=np.float32)
    for b in range(B):
        acc = results[b * 4 + 0]["out_p"].astype(np.float64)
        for g in range(1, 4):
            acc += results[b * 4 + g]["out_p"]
        out[b] = (acc + bo).astype(np.float32)
    return out


def _get_runner():
    """Build (once) a jitted shard_map callable executing the compiled
    kernel on 8 cores. Adapted from concourse.bass2jax.run_bass_via_pjrt,
    minus output-buffer donation so the callable is re-invokable for
    timing."""
    if "runner" in _CACHE:
        return _CACHE["runner"]
    import jax
    from jax.experimental.shard_map import shard_map
    from jax.sharding import Mesh, PartitionSpec
    from concourse import bass2jax
    from concourse.bass2jax import _bass_exec_p, install_neuronx_cc_hook

    install_neuronx_cc_hook()
    nc = build_nc()
    partition_name = (
        nc.partition_id_tensor.name if nc.partition_id_tensor else None
    )

    in_names, out_names, out_avals, zero_outs = [], [], [], []
    for alloc in nc.m.functions[0].allocations:
        if not isinstance(alloc, mybir.MemoryLocationSet):
            continue
        name = alloc.memorylocations[0].name
        if alloc.kind == "ExternalInput":
            if name != partition_name:
                in_names.append(name)
        elif alloc.kind == "ExternalOutput":
            out_names.append(name)
            shape = tuple(alloc.tensor_shape)
            dtype = mybir.dt.np(alloc.dtype)
            out_avals.append(jax.core.ShapedArray(shape, dtype))
            zero_outs.append(np.zeros(shape, dtype))
    n_params = len(in_names)
    all_names = in_names + out_names
    if partition_name is not None:
        all_names = all_names + [partition_name]

    def _body(*args):
        operands = list(args)
        if partition_name is not None:
            operands.append(bass2jax.partition_id_tensor())
        outs = _bass_exec_p.bind(
            *operands,
            out_avals=tuple(out_avals),
            in_names=tuple(all_names),
            out_names=tuple(out_names),
            lowering_input_output_aliases=(),
            sim_require_finite=True,
            sim_require_nnan=True,
            nc=nc,
        )
        return tuple(outs)

    devices = jax.devices()[:N_CORES]
    mesh = Mesh(np.asarray(devices), ("core",))
    n_all = n_params + len(out_names)
    sharded = jax.jit(
        shard_map(
            _body,
            mesh=mesh,
            in_specs=(PartitionSpec("core"),) * n_all,
            out_specs=(PartitionSpec("core"),) * len(out_names),
            check_rep=False,
        ),
        keep_unused=True,
    )
    runner = {
        "sharded": sharded,
        "in_names": in_names,
        "out_names": out_names,
        "out_avals": out_avals,
        "zero_outs": zero_outs,
        "mesh": mesh,
        "nc": nc,
    }
    _CACHE["runner"] = runner
    return runner


def _run(in_maps):
    r = _get_runner()
    concat_in = [
        np.concatenate([np.asarray(in_maps[c][n]) for c in range(N_CORES)], axis=0)
        for n in r["in_names"]
    ]
    concat_zeros = [
        np.zeros((N_CORES * z.shape[0], *z.shape[1:]), z.dtype)
        for z in r["zero_outs"]
    ]
    out_arrs = r["sharded"](*concat_in, *concat_zeros)
    _CACHE["last_args"] = (concat_in, concat_zeros)
    return [
        {
            n: np.asarray(out_arrs[i]).reshape(
                N_CORES, *r["out_avals"][i].shape
            )[c]
            for i, n in enumerate(r["out_names"])
        }
        for c in range(N_CORES)
    ]


def bench(iters=10):
    """Re-execute the last-run kernel with device-resident inputs and
    return per-call wall times (s). Outputs stay on device."""
    import time as _time
    import jax
    from jax.sharding import NamedSharding, PartitionSpec

    r = _CACHE["runner"]
    concat_in, concat_zeros = _CACHE["last_args"]
    sh = NamedSharding(r["mesh"], PartitionSpec("core"))
    dev_args = [jax.device_put(a, sh) for a in (*concat_in, *concat_zeros)]
    for a in dev_args:
        a.block_until_ready()
    times = []
    for _ in range(iters):
        t0 = _time.perf_counter()
        outs = r["sharded"](*dev_args)
        for o in outs:
            o.block_until_ready()
        times.append(_time.perf_counter() - t0)
    return times


# revision 40
# speedup vs baseline: 363.5295x; 1.1340x over previous
"""GQA attention kernel for 8 Trainium2 NeuronCores.

Problem: B=2, S=2048, D=1024, 16 Q heads / 4 KV heads (GQA), causal,
y = softmax((x@wq+bq)(x@wk+bk)^T / 8, causal) @ (x@wv+bv) @ wo + bo

Sharding: core c -> (batch b = c//4, kv-group g = c%4). Each core computes
its batch's attention for 4 Q heads (= 1 KV head) and the partial output
projection through wo[g*256:(g+1)*256, :]. Host sums the 4 partials per
batch and adds bo.

Device kernel layout (v2):
  - x arrives pre-transposed from host as xT [D, S] bf16: no PE transposes.
  - kT [128, S]: rows 0-63 = k^T, rows 64-127 = duplicate (SBUF->SBUF DMA),
    so the PE can run in 64x128 row-tiled mode with two concurrent tiles
    (T0 = SBUF partitions 0-63, T8 = partitions 64-127).
  - qT per head-pair [128, S]: rows 0-63 = even head, 64-127 = odd head.
  - scores for the two heads of a pair run CONCURRENTLY on T0/T8.
  - AV contraction (128 keys) is split into two 64-key halves on T0/T8
    accumulating into separate PSUM banks (accA/accB); summed during the
    softmax-normalize step on DVE. No PE mode switches inside attention.
  - softmax denominator comes from a ones-column appended to v; 1/den via
    DVE reciprocal_approx_fast, partition-broadcast on gpsimd, applied on
    DVE. exp() width-trimmed on causal-diagonal tiles.
"""

import sys
from contextlib import ExitStack

import numpy as np
import ml_dtypes

if "/opt/trn_rl_repo" not in sys.path:
    sys.path.insert(0, "/opt/trn_rl_repo")

import concourse.bass as bass
import concourse.tile as tile
from concourse import bacc, mybir
from concourse.masks import make_identity

B, S, D = 2, 2048, 1024
H, KVH, HD = 16, 4, 64
GQ = H // KVH        # 4 q heads per core
DG = GQ * HD         # 256 q dims per core
P = 128
KC = D // P          # 8 contraction chunks over D
NKT = S // P         # 16 key tiles
NQB = S // 512       # 4 query blocks
N_CORES = 8

DT = mybir.dt.float32
DTB = mybir.dt.bfloat16
AF = mybir.ActivationFunctionType
BF16 = ml_dtypes.bfloat16

_CACHE = {}

import os
# bisect flags (HW debug)
ROW_TILED = os.environ.get("K_ROW_TILED", "1") == "1"
# reciprocal_approx_fast (custom DVE ucode) and gpsimd partition_broadcast
# both produce garbage on HW through this execution path (sim is fine) —
# keep them off; plain DVE reciprocal + ones-row matmul broadcast work.
APPROX_RECIP = os.environ.get("K_APPROX_RECIP", "0") == "1"
PBCAST = os.environ.get("K_PBCAST", "0") == "1"
FILL = os.environ.get("K_FILL", "1") == "1"
# software-pipelined kt loop: scores run one step ahead of AV so the exp
# latency is covered by useful PE work; AV is full-K (single acc per head)
# freeing two PSUM banks for the deeper scores pipeline.
PIPE = os.environ.get("K_PIPE", "1") == "1"
# 1/den via exp(-ln(den)) on ACT: ln+exp live in the same activation table
# set (no table thrash), batched per pair as [1,1024] — replaces the 3.4us
# per-head DVE reciprocal.
LNRECIP = os.environ.get("K_LNRECIP", "1") == "1"


def build_nc():
    nc = bacc.Bacc(
        "TRN2",
        target_bir_lowering=False,
        debug=False,
        enable_asserts=False,
        num_devices=N_CORES,
    )
    xtd = nc.dram_tensor("xtd", [D, S], DTB, kind="ExternalInput").ap()
    wqd = nc.dram_tensor("wqd", [D, DG], DTB, kind="ExternalInput").ap()
    wkvd = nc.dram_tensor("wkvd", [D, 2 * HD], DTB, kind="ExternalInput").ap()
    wod = nc.dram_tensor("wod", [DG, D], DTB, kind="ExternalInput").ap()
    bqd = nc.dram_tensor("bqd", [DG, 1], DT, kind="ExternalInput").ap()
    bkvd = nc.dram_tensor("bkvd", [2 * HD, 1], DT, kind="ExternalInput").ap()
    out_p = nc.dram_tensor("out_p", [S, D], DT, kind="ExternalOutput").ap()

    with tile.TileContext(nc) as tc, ExitStack() as ctx:
        consts = ctx.enter_context(tc.tile_pool(name="consts", bufs=1))
        etp = ctx.enter_context(tc.tile_pool(name="etp", bufs=6))
        vtp = ctx.enter_context(tc.tile_pool(name="vtp", bufs=2))
        dnp = ctx.enter_context(tc.tile_pool(name="dnp", bufs=3))
        rbp = ctx.enter_context(tc.tile_pool(name="rbp", bufs=3))
        ysb = ctx.enter_context(tc.tile_pool(name="ysb", bufs=3))
        psP = ctx.enter_context(tc.tile_pool(name="psP", bufs=2, space="PSUM"))
        psS = ctx.enter_context(
            tc.tile_pool(name="psS", bufs=(4 if PIPE else 2), space="PSUM"))
        psA = ctx.enter_context(
            tc.tile_pool(name="psA", bufs=(2 if PIPE else 4), space="PSUM"))

        # persistent SBUF
        xT = [consts.tile([P, S], DTB, tag=f"xT{kc}", name=f"xT{kc}") for kc in range(KC)]
        kT = consts.tile([P, S], DTB, tag="kT")
        qT = [consts.tile([P, S], DTB, tag=f"qT{pr}", name=f"qT{pr}") for pr in range(2)]
        qTlo = (None if ROW_TILED else
                [consts.tile([HD, S], DTB, tag=f"qTlo{pr}", name=f"qTlo{pr}")
                 for pr in range(2)])
        vA = consts.tile([P, NKT, HD + 1], DTB, tag="vA")
        oT = [consts.tile([P, S], DTB, tag=f"oT{c}", name=f"oT{c}") for c in range(2)]
        wq_sb = [consts.tile([P, DG], DTB, tag=f"wq{kc}", name=f"wq{kc}") for kc in range(KC)]
        wkv_sb = [consts.tile([P, 2 * HD], DTB, tag=f"wkv{kc}", name=f"wkv{kc}") for kc in range(KC)]
        wo_sb = [consts.tile([P, D], DTB, tag=f"wo{c}", name=f"wo{c}") for c in range(2)]
        bq_sb = consts.tile([P, 2], DT, tag="bq")
        # rows 0-63 = bk (aligned with k's PSUM rows), 64-127 = bv
        bkv_sb = consts.tile([P, 1], DT, tag="bkv")
        ident = consts.tile([HD, HD], DTB, tag="ident")
        ones_row = consts.tile([1, HD], DTB, tag="ones")
        nc.vector.memset(ones_row, 1.0)

        # ---- DMA queues: sync/gpsimd/scalar can issue DMA; scalar only
        # used for prologue loads (ACT is exp-bound in steady state) ----
        QS = [nc.sync, nc.gpsimd, nc.scalar]
        qi = 0

        def dq():
            nonlocal qi
            qi += 1
            return QS[qi % len(QS)]

        # identity for the small v transposes
        make_identity(nc, ident)

        for kc in range(KC):
            dq().dma_start(wq_sb[kc], wqd[kc * P:(kc + 1) * P, :])
            dq().dma_start(wkv_sb[kc], wkvd[kc * P:(kc + 1) * P, :])
        for c in range(2):
            dq().dma_start(wo_sb[c], wod[c * P:(c + 1) * P, :])
        for mc in range(2):
            dq().dma_start(bq_sb[:, mc:mc + 1], bqd[mc * P:(mc + 1) * P, :])
        dq().dma_start(bkv_sb, bkvd[:, :])
        # x^T tiles: first s-block separately so projections start early
        for kc in range(KC):
            dq().dma_start(xT[kc][:, 0:512], xtd[kc * P:(kc + 1) * P, 0:512])
        for kc in range(KC):
            dq().dma_start(xT[kc][:, 512:S], xtd[kc * P:(kc + 1) * P, 512:S])
        nc.vector.memset(vA[:, :, HD:HD + 1], 1.0)

        def proj_kv(nb):
            sl = slice(nb * 512, (nb + 1) * 512)
            # k|v fused projection
            ps2 = psP.tile([P, 512], DT, tag="pp", name="ps2")
            for kc in range(KC):
                nc.tensor.matmul(ps2, wkv_sb[kc], xT[kc][:, sl],
                                 start=(kc == 0), stop=(kc == KC - 1))
            nc.vector.tensor_scalar_add(kT[0:HD, sl], ps2[0:HD, :], bkv_sb[0:HD, :])
            nc.sync.dma_start(kT[HD:P, sl], kT[0:HD, sl])
            vt = vtp.tile([HD, 512], DTB, tag="vt", name="vt")
            nc.vector.tensor_scalar_add(
                vt, ps2[HD:2 * HD, :], bkv_sb[HD:P, :])
            return vt

        def proj_vtrans(nb, vt):
            for j in range(4):
                kt_i = nb * 4 + j
                vps = psP.tile([P, HD], DTB, tag="pp", name="vps")
                nc.tensor.transpose(vps, vt[:, j * P:(j + 1) * P], ident)
                nc.vector.tensor_copy(vA[:, kt_i, 0:HD], vps)

        def proj_q(nb, mc):
            sl = slice(nb * 512, (nb + 1) * 512)
            ps = psP.tile([P, 512], DT, tag="pp", name="psq")
            for kc in range(KC):
                nc.tensor.matmul(
                    ps, wq_sb[kc][:, mc * P:(mc + 1) * P], xT[kc][:, sl],
                    start=(kc == 0), stop=(kc == KC - 1))
            nc.vector.tensor_scalar(
                out=qT[mc][:, sl], in0=ps,
                scalar1=0.125, scalar2=bq_sb[:, mc:mc + 1],
                op0=mybir.AluOpType.mult, op1=mybir.AluOpType.add)
            if not ROW_TILED:
                nc.vector.tensor_copy(qTlo[mc][:, sl], qT[mc][HD:P, sl])

        def proj_pieces(nb):
            state = {}

            def p0():
                state["vt"] = proj_kv(nb)

            return [p0,
                    lambda: proj_vtrans(nb, state["vt"]),
                    lambda: proj_q(nb, 0),
                    lambda: proj_q(nb, 1)]

        def attn_pair_pipe(qb, pr, fill_iter=None):
            """Software-pipelined attention for head pair pr: scores/exp for
            kt are emitted one step ahead of AV(kt-1), so the PE never
            head-of-line blocks on the exp of the tile it is about to
            consume. Scores run row-tile-concurrent (T0/T8); AV is full-K."""
            qsl0 = qb * 512
            nkt = 4 * (qb + 1)
            acc = [psA.tile([HD + 1, 512], DT, tag="acc", name="acc") for _ in range(2)]

            def emit_scores_exp(kt):
                di = kt - 4 * qb
                off = di * P if di > 0 else 0
                wv_ = 512 - off
                ktsl = slice(kt * P, (kt + 1) * P)
                qssl = slice(qsl0 + off, qsl0 + 512)
                s0 = psS.tile([P, 512], DT, tag="ss", name="s0")
                s1 = psS.tile([P, 512], DT, tag="ss", name="s1")
                nc.tensor.matmul(s0[:, off:512], kT[0:HD, ktsl], qT[pr][0:HD, qssl],
                                 start=True, stop=True)
                nc.tensor.matmul(s1[:, off:512], kT[HD:P, ktsl], qT[pr][HD:P, qssl],
                                 start=True, stop=True)
                et0 = etp.tile([P, 512], DTB, tag="et", name="et0")
                et1 = etp.tile([P, 512], DTB, tag="et", name="et1")
                nc.scalar.activation(et0[:, off:512], s0[:, off:512], AF.Exp)
                nc.scalar.activation(et1[:, off:512], s1[:, off:512], AF.Exp)
                if di >= 0:
                    base = qb * 512 + off - kt * P
                    nc.gpsimd.affine_select(
                        out=et0[:, off:512], in_=et0[:, off:512],
                        pattern=[[1, wv_]],
                        compare_op=mybir.AluOpType.is_ge, fill=0.0,
                        base=base, channel_multiplier=-1)
                    nc.gpsimd.affine_select(
                        out=et1[:, off:512], in_=et1[:, off:512],
                        pattern=[[1, wv_]],
                        compare_op=mybir.AluOpType.is_ge, fill=0.0,
                        base=base, channel_multiplier=-1)
                return off, et0, et1

            def emit_av(kt, off, et0, et1):
                st = (kt == 0)
                sp = (kt == nkt - 1)
                o5 = slice(off, 512)
                nc.tensor.matmul(acc[0][:, o5], vA[:, kt, :], et0[:, o5], start=st, stop=sp)
                nc.tensor.matmul(acc[1][:, o5], vA[:, kt, :], et1[:, o5], start=st, stop=sp)

            prev = None
            for kt in range(nkt):
                off, et0, et1 = emit_scores_exp(kt)
                if prev is not None:
                    emit_av(*prev)
                    if fill_iter is not None and kt % 2 == 0:
                        f = next(fill_iter, None)
                        if f is not None:
                            f()
                prev = (kt, off, et0, et1)
            emit_av(*prev)

            qsl = slice(qsl0, qsl0 + 512)
            for hh in range(2):
                h = 2 * pr + hh
                c, r0 = h // 2, (h % 2) * HD
                dens = dnp.tile([HD + 1, 512], DT, tag="dens", name="dens")
                nc.vector.tensor_copy(dens[HD:HD + 1, :], acc[hh][HD:HD + 1, :])
                rec = dnp.tile([1, 512], DT, tag="rec", name="rec")
                nc.vector.reciprocal(rec, dens[HD:HD + 1, :])
                recb = dnp.tile([1, 512], DTB, tag="recb", name="recb")
                nc.vector.tensor_copy(recb, rec)
                recB = rbp.tile([P, 512], DTB, tag="recB", name="recB")
                rbps = psP.tile([HD, 512], DT, tag="pp", name="rbps")
                nc.tensor.matmul(rbps, ones_row, recb, start=True, stop=True)
                nc.vector.tensor_copy(recB[r0:r0 + HD, :], rbps)
                nc.vector.tensor_mul(
                    oT[c][r0:r0 + HD, qsl], acc[hh][0:HD, :], recB[r0:r0 + HD, :])

        def attn_pair(qb, pr, fill_iter=None):
            """Attention for head pair pr (local heads 2pr, 2pr+1), q block qb."""
            qsl0 = qb * 512
            nkt = 4 * (qb + 1)
            accA = [psA.tile([HD + 1, 512], DT, tag="acc", name="accA") for _ in range(2)]
            accB = (
                [psA.tile([HD + 1, 512], DT, tag="acc", name="accB") for _ in range(2)]
                if ROW_TILED else [None, None])
            for kt in range(nkt):
                di = kt - 4 * qb
                off = di * P if di > 0 else 0
                wv_ = 512 - off
                ktsl = slice(kt * P, (kt + 1) * P)
                qssl = slice(qsl0 + off, qsl0 + 512)
                s0 = psS.tile([P, 512], DT, tag="ss", name="s0")
                s1 = psS.tile([P, 512], DT, tag="ss", name="s1")
                nc.tensor.matmul(s0[:, off:512], kT[0:HD, ktsl], qT[pr][0:HD, qssl],
                                 start=True, stop=True)
                if ROW_TILED:
                    nc.tensor.matmul(s1[:, off:512], kT[HD:P, ktsl], qT[pr][HD:P, qssl],
                                     start=True, stop=True)
                else:
                    nc.tensor.matmul(s1[:, off:512], kT[0:HD, ktsl], qTlo[pr][:, qssl],
                                     start=True, stop=True)
                et0 = etp.tile([P, 512], DTB, tag="et", name="et0")
                et1 = etp.tile([P, 512], DTB, tag="et", name="et1")
                nc.scalar.activation(et0[:, off:512], s0[:, off:512], AF.Exp)
                nc.scalar.activation(et1[:, off:512], s1[:, off:512], AF.Exp)
                if di >= 0:
                    # causal: keep where (qb*512 + off + f) - p - kt*128 >= 0,
                    # i.e. q_abs >= key_abs; the AP starts at column `off` so
                    # fold it into base.
                    base = qb * 512 + off - kt * P
                    nc.gpsimd.affine_select(
                        out=et0[:, off:512], in_=et0[:, off:512],
                        pattern=[[1, wv_]],
                        compare_op=mybir.AluOpType.is_ge, fill=0.0,
                        base=base, channel_multiplier=-1)
                    nc.gpsimd.affine_select(
                        out=et1[:, off:512], in_=et1[:, off:512],
                        pattern=[[1, wv_]],
                        compare_op=mybir.AluOpType.is_ge, fill=0.0,
                        base=base, channel_multiplier=-1)
                if fill_iter is not None and kt % 2 == 1:
                    # interleave a projection/output-proj piece: the PE chews
                    # on it while ACT runs the exps, instead of idling.
                    f = next(fill_iter, None)
                    if f is not None:
                        f()
                st = (kt == 0)
                sp = (kt == nkt - 1)
                o5 = slice(off, 512)
                if ROW_TILED:
                    # split-K AV: T0 half (keys 0-63) and T8 half (keys
                    # 64-127) run concurrently; consecutive instructions
                    # alternate banks.
                    nc.tensor.matmul(accA[0][:, o5], vA[0:HD, kt, :], et0[0:HD, o5], start=st, stop=sp)
                    nc.tensor.matmul(accB[1][:, o5], vA[HD:P, kt, :], et1[HD:P, o5], start=st, stop=sp)
                    nc.tensor.matmul(accB[0][:, o5], vA[HD:P, kt, :], et0[HD:P, o5], start=st, stop=sp)
                    nc.tensor.matmul(accA[1][:, o5], vA[0:HD, kt, :], et1[0:HD, o5], start=st, stop=sp)
                else:
                    nc.tensor.matmul(accA[0][:, o5], vA[:, kt, :], et0[:, o5], start=st, stop=sp)
                    nc.tensor.matmul(accA[1][:, o5], vA[:, kt, :], et1[:, o5], start=st, stop=sp)
            qsl = slice(qsl0, qsl0 + 512)
            for hh in range(2):
                h = 2 * pr + hh
                c, r0 = h // 2, (h % 2) * HD
                dens = dnp.tile([HD + 1, 512], DT, tag="dens", name="dens")
                if ROW_TILED:
                    # only one PSUM operand per DVE op: stage accB in SBUF
                    obf = dnp.tile([HD + 1, 512], DT, tag="obf", name="obf")
                    nc.vector.tensor_copy(obf, accB[hh])
                    nc.vector.tensor_add(
                        dens[HD:HD + 1, :], accA[hh][HD:HD + 1, :], obf[HD:HD + 1, :])
                else:
                    nc.vector.tensor_copy(dens[HD:HD + 1, :], accA[hh][HD:HD + 1, :])
                recb = dnp.tile([1, 512], DTB, tag="recb", name="recb")
                if APPROX_RECIP:
                    # custom-DVE op: keep in/out partition-aligned (row 64)
                    rec = dnp.tile([HD + 1, 512], DT, tag="rec", name="rec")
                    nc.vector.reciprocal_approx_fast(
                        out=rec[HD:HD + 1, :], in_=dens[HD:HD + 1, :])
                    nc.vector.tensor_copy(recb, rec[HD:HD + 1, :])
                else:
                    rec = dnp.tile([1, 512], DT, tag="rec", name="rec")
                    nc.vector.reciprocal(rec, dens[HD:HD + 1, :])
                    nc.vector.tensor_copy(recb, rec)
                recB = rbp.tile([P, 512], DTB, tag="recB", name="recB")
                if PBCAST:
                    nc.gpsimd.partition_broadcast(
                        recB[r0:r0 + HD, :], recb, channels=HD)
                else:
                    # broadcast 1/den across 64 head dims via a K=1 matmul;
                    # psP (not psS) so the next pair's scores aren't blocked
                    rbps = psP.tile([HD, 512], DT, tag="pp", name="rbps")
                    nc.tensor.matmul(rbps, ones_row, recb, start=True, stop=True)
                    nc.vector.tensor_copy(recB[r0:r0 + HD, :], rbps)
                if ROW_TILED:
                    nc.vector.tensor_add(
                        oT[c][r0:r0 + HD, qsl], accA[hh][0:HD, :], obf[0:HD, :])
                    nc.vector.tensor_mul(
                        oT[c][r0:r0 + HD, qsl], oT[c][r0:r0 + HD, qsl], recB[r0:r0 + HD, :])
                else:
                    nc.vector.tensor_mul(
                        oT[c][r0:r0 + HD, qsl], accA[hh][0:HD, :], recB[r0:r0 + HD, :])

        def wo_st(st):
            yt = ysb.tile([P, D], DT, tag="y", name="yt")
            for nb2 in range(2):
                yps = psP.tile([P, 512], DT, tag="pp", name="yps")
                for c in range(2):
                    nc.tensor.matmul(
                        yps, oT[c][:, st * P:(st + 1) * P],
                        wo_sb[c][:, nb2 * 512:(nb2 + 1) * 512],
                        start=(c == 0), stop=(c == 1))
                nc.vector.tensor_copy(yt[:, nb2 * 512:(nb2 + 1) * 512], yps)
            QS[st % 2].dma_start(out_p[st * P:(st + 1) * P, :], yt)

        def wo_pieces(bl):
            return [(lambda st=st: wo_st(st)) for st in range(bl * 4, bl * 4 + 4)]

        attn = attn_pair_pipe if PIPE else attn_pair

        if FILL:
            for f in proj_pieces(0):
                f()
            for qb in range(4):
                fillers = []
                if qb + 1 < 4:
                    fillers += proj_pieces(qb + 1)
                if qb >= 1:
                    fillers += wo_pieces(qb - 1)
                fill_iter = iter(fillers)
                attn(qb, 0, fill_iter)
                attn(qb, 1, fill_iter)
                for f in fill_iter:
                    f()
            for f in wo_pieces(3):
                f()
        else:
            for nb in range(4):
                for f in proj_pieces(nb):
                    f()
                if nb >= 1:
                    for f in wo_pieces(nb - 1):
                        f()
                attn(nb, 0)
                attn(nb, 1)
            for f in wo_pieces(3):
                f()

    nc.compile()
    return nc


def prepare_in_maps(x, wq, bq, wk, bk, wv, bv, wo):
    x = np.asarray(x, dtype=np.float32)
    xb = [np.ascontiguousarray(x[b].T).astype(BF16) for b in range(B)]
    wqb = np.asarray(wq, dtype=np.float32).astype(BF16)
    wkb = np.asarray(wk, dtype=np.float32).astype(BF16)
    wvb = np.asarray(wv, dtype=np.float32).astype(BF16)
    wob = np.asarray(wo, dtype=np.float32).astype(BF16)
    bq = np.asarray(bq, dtype=np.float32)
    bk = np.asarray(bk, dtype=np.float32)
    bv = np.asarray(bv, dtype=np.float32)

    in_maps = []
    for c in range(N_CORES):
        b, g = c // 4, c % 4
        sq = slice(g * DG, (g + 1) * DG)
        sk = slice(g * HD, (g + 1) * HD)
        in_maps.append({
            "xtd": xb[b],
            "wqd": np.ascontiguousarray(wqb[:, sq]),
            "wkvd": np.ascontiguousarray(
                np.concatenate([wkb[:, sk], wvb[:, sk]], axis=1)),
            "wod": np.ascontiguousarray(wob[sq, :]),
            "bqd": np.ascontiguousarray((bq[sq] * 0.125).reshape(DG, 1)),
            "bkvd": np.ascontiguousarray(
                np.concatenate([bk[sk], bv[sk]]).reshape(2 * HD, 1)),
        })
    return in_maps


def kernel(x, mask, wq, bq, wk, bk, wv, bv, wo, bo):
    bo = np.asarray(bo, dtype=np.float32)
    in_maps = prepare_in_maps(x, wq, bq, wk, bk, wv, bv, wo)
    results = _run(in_maps)

    out = np.empty((B, S, D), dtype=np.float32)
    for b in range(B):
        acc = results[b * 4 + 0]["out_p"].astype(np.float64)
        for g in range(1, 4):
            acc += results[b * 4 + g]["out_p"]
        out[b] = (acc + bo).astype(np.float32)
    return out


def _get_runner():
    """Build (once) a jitted shard_map callable executing the compiled
    kernel on 8 cores. Adapted from concourse.bass2jax.run_bass_via_pjrt,
    minus output-buffer donation so the callable is re-invokable for
    timing."""
    if "runner" in _CACHE:
        return _CACHE["runner"]
    import jax
    from jax.experimental.shard_map import shard_map
    from jax.sharding import Mesh, PartitionSpec
    from concourse import bass2jax
    from concourse.bass2jax import _bass_exec_p, install_neuronx_cc_hook

    install_neuronx_cc_hook()
    nc = build_nc()
    partition_name = (
        nc.partition_id_tensor.name if nc.partition_id_tensor else None
    )

    in_names, out_names, out_avals, zero_outs = [], [], [], []
    for alloc in nc.m.functions[0].allocations:
        if not isinstance(alloc, mybir.MemoryLocationSet):
            continue
        name = alloc.memorylocations[0].name
        if alloc.kind == "ExternalInput":
            if name != partition_name:
                in_names.append(name)
        elif alloc.kind == "ExternalOutput":
            out_names.append(name)
            shape = tuple(alloc.tensor_shape)
            dtype = mybir.dt.np(alloc.dtype)
            out_avals.append(jax.core.ShapedArray(shape, dtype))
            zero_outs.append(np.zeros(shape, dtype))
    n_params = len(in_names)
    all_names = in_names + out_names
    if partition_name is not None:
        all_names = all_names + [partition_name]

    def _body(*args):
        operands = list(args)
        if partition_name is not None:
            operands.append(bass2jax.partition_id_tensor())
        outs = _bass_exec_p.bind(
            *operands,
            out_avals=tuple(out_avals),
            in_names=tuple(all_names),
            out_names=tuple(out_names),
            lowering_input_output_aliases=(),
            sim_require_finite=True,
            sim_require_nnan=True,
            nc=nc,
        )
        return tuple(outs)

    devices = jax.devices()[:N_CORES]
    mesh = Mesh(np.asarray(devices), ("core",))
    n_all = n_params + len(out_names)
    sharded = jax.jit(
        shard_map(
            _body,
            mesh=mesh,
            in_specs=(PartitionSpec("core"),) * n_all,
            out_specs=(PartitionSpec("core"),) * len(out_names),
            check_rep=False,
        ),
        keep_unused=True,
    )
    runner = {
        "sharded": sharded,
        "in_names": in_names,
        "out_names": out_names,
        "out_avals": out_avals,
        "zero_outs": zero_outs,
        "mesh": mesh,
        "nc": nc,
    }
    _CACHE["runner"] = runner
    return runner


def _run(in_maps):
    r = _get_runner()
    concat_in = [
        np.concatenate([np.asarray(in_maps[c][n]) for c in range(N_CORES)], axis=0)
        for n in r["in_names"]
    ]
    concat_zeros = [
        np.zeros((N_CORES * z.shape[0], *z.shape[1:]), z.dtype)
        for z in r["zero_outs"]
    ]
    out_arrs = r["sharded"](*concat_in, *concat_zeros)
    _CACHE["last_args"] = (concat_in, concat_zeros)
    return [
        {
            n: np.asarray(out_arrs[i]).reshape(
                N_CORES, *r["out_avals"][i].shape
            )[c]
            for i, n in enumerate(r["out_names"])
        }
        for c in range(N_CORES)
    ]


def bench(iters=10):
    """Re-execute the last-run kernel with device-resident inputs and
    return per-call wall times (s). Outputs stay on device."""
    import time as _time
    import jax
    from jax.sharding import NamedSharding, PartitionSpec

    r = _CACHE["runner"]
    concat_in, concat_zeros = _CACHE["last_args"]
    sh = NamedSharding(r["mesh"], PartitionSpec("core"))
    dev_args = [jax.device_put(a, sh) for a in (*concat_in, *concat_zeros)]
    for a in dev_args:
        a.block_until_ready()
    times = []
    for _ in range(iters):
        t0 = _time.perf_counter()
        outs = r["sharded"](*dev_args)
        for o in outs:
            o.block_until_ready()
        times.append(_time.perf_counter() - t0)
    return times


# revision 42
# speedup vs baseline: 385.4534x; 1.0603x over previous
"""GQA attention kernel for 8 Trainium2 NeuronCores.

Problem: B=2, S=2048, D=1024, 16 Q heads / 4 KV heads (GQA), causal,
y = softmax((x@wq+bq)(x@wk+bk)^T / 8, causal) @ (x@wv+bv) @ wo + bo

Sharding: core c -> (batch b = c//4, kv-group g = c%4). Each core computes
its batch's attention for 4 Q heads (= 1 KV head) and the partial output
projection through wo[g*256:(g+1)*256, :]. Host sums the 4 partials per
batch and adds bo.

Device kernel layout (v2):
  - x arrives pre-transposed from host as xT [D, S] bf16: no PE transposes.
  - kT [128, S]: rows 0-63 = k^T, rows 64-127 = duplicate (SBUF->SBUF DMA),
    so the PE can run in 64x128 row-tiled mode with two concurrent tiles
    (T0 = SBUF partitions 0-63, T8 = partitions 64-127).
  - qT per head-pair [128, S]: rows 0-63 = even head, 64-127 = odd head.
  - scores for the two heads of a pair run CONCURRENTLY on T0/T8.
  - AV contraction (128 keys) is split into two 64-key halves on T0/T8
    accumulating into separate PSUM banks (accA/accB); summed during the
    softmax-normalize step on DVE. No PE mode switches inside attention.
  - softmax denominator comes from a ones-column appended to v; 1/den via
    DVE reciprocal_approx_fast, partition-broadcast on gpsimd, applied on
    DVE. exp() width-trimmed on causal-diagonal tiles.
"""

import sys
from contextlib import ExitStack

import numpy as np
import ml_dtypes

if "/opt/trn_rl_repo" not in sys.path:
    sys.path.insert(0, "/opt/trn_rl_repo")

import concourse.bass as bass
import concourse.tile as tile
from concourse import bacc, mybir
from concourse.masks import make_identity

B, S, D = 2, 2048, 1024
H, KVH, HD = 16, 4, 64
GQ = H // KVH        # 4 q heads per core
DG = GQ * HD         # 256 q dims per core
P = 128
KC = D // P          # 8 contraction chunks over D
NKT = S // P         # 16 key tiles
NQB = S // 512       # 4 query blocks
N_CORES = 8

DT = mybir.dt.float32
DTB = mybir.dt.bfloat16
AF = mybir.ActivationFunctionType
BF16 = ml_dtypes.bfloat16

_CACHE = {}

import os
# bisect flags (HW debug)
ROW_TILED = os.environ.get("K_ROW_TILED", "1") == "1"
# reciprocal_approx_fast (custom DVE ucode) and gpsimd partition_broadcast
# both produce garbage on HW through this execution path (sim is fine) —
# keep them off; plain DVE reciprocal + ones-row matmul broadcast work.
APPROX_RECIP = os.environ.get("K_APPROX_RECIP", "0") == "1"
PBCAST = os.environ.get("K_PBCAST", "0") == "1"
FILL = os.environ.get("K_FILL", "1") == "1"
# software-pipelined kt loop: scores run one step ahead of AV so the exp
# latency is covered by useful PE work; AV is full-K (single acc per head)
# freeing two PSUM banks for the deeper scores pipeline.
PIPE = os.environ.get("K_PIPE", "1") == "1"
# 1/den via exp(-ln(den)) on ACT: ln+exp live in the same activation table
# set (no table thrash), batched per pair as [1,1024] — replaces the 3.4us
# per-head DVE reciprocal.
LNRECIP = os.environ.get("K_LNRECIP", "1") == "1"


def build_nc():
    nc = bacc.Bacc(
        "TRN2",
        target_bir_lowering=False,
        debug=False,
        enable_asserts=False,
        num_devices=N_CORES,
    )
    xtd = nc.dram_tensor("xtd", [D, S], DTB, kind="ExternalInput").ap()
    wqd = nc.dram_tensor("wqd", [D, DG], DTB, kind="ExternalInput").ap()
    wkvd = nc.dram_tensor("wkvd", [D, 2 * HD], DTB, kind="ExternalInput").ap()
    wod = nc.dram_tensor("wod", [DG, D], DTB, kind="ExternalInput").ap()
    bqd = nc.dram_tensor("bqd", [DG, 1], DT, kind="ExternalInput").ap()
    bkvd = nc.dram_tensor("bkvd", [2 * HD, 1], DT, kind="ExternalInput").ap()
    out_p = nc.dram_tensor("out_p", [S, D], DT, kind="ExternalOutput").ap()

    with tile.TileContext(nc) as tc, ExitStack() as ctx:
        consts = ctx.enter_context(tc.tile_pool(name="consts", bufs=1))
        etp = ctx.enter_context(tc.tile_pool(name="etp", bufs=6))
        vtp = ctx.enter_context(tc.tile_pool(name="vtp", bufs=2))
        dnp = ctx.enter_context(tc.tile_pool(name="dnp", bufs=3))
        rbp = ctx.enter_context(tc.tile_pool(name="rbp", bufs=3))
        ysb = ctx.enter_context(tc.tile_pool(name="ysb", bufs=3))
        psP = ctx.enter_context(tc.tile_pool(name="psP", bufs=2, space="PSUM"))
        psS = ctx.enter_context(
            tc.tile_pool(name="psS", bufs=(4 if PIPE else 2), space="PSUM"))
        psA = ctx.enter_context(
            tc.tile_pool(name="psA", bufs=(2 if PIPE else 4), space="PSUM"))

        # persistent SBUF
        xT = [consts.tile([P, S], DTB, tag=f"xT{kc}", name=f"xT{kc}") for kc in range(KC)]
        kT = consts.tile([P, S], DTB, tag="kT")
        qT = [consts.tile([P, S], DTB, tag=f"qT{pr}", name=f"qT{pr}") for pr in range(2)]
        qTlo = (None if ROW_TILED else
                [consts.tile([HD, S], DTB, tag=f"qTlo{pr}", name=f"qTlo{pr}")
                 for pr in range(2)])
        vA = consts.tile([P, NKT, HD + 1], DTB, tag="vA")
        oT = [consts.tile([P, S], DTB, tag=f"oT{c}", name=f"oT{c}") for c in range(2)]
        wq_sb = [consts.tile([P, DG], DTB, tag=f"wq{kc}", name=f"wq{kc}") for kc in range(KC)]
        wkv_sb = [consts.tile([P, 2 * HD], DTB, tag=f"wkv{kc}", name=f"wkv{kc}") for kc in range(KC)]
        wo_sb = [consts.tile([P, D], DTB, tag=f"wo{c}", name=f"wo{c}") for c in range(2)]
        bq_sb = consts.tile([P, 2], DT, tag="bq")
        # rows 0-63 = bk (aligned with k's PSUM rows), 64-127 = bv
        bkv_sb = consts.tile([P, 1], DT, tag="bkv")
        ident = consts.tile([HD, HD], DTB, tag="ident")
        ones_row = consts.tile([1, HD], DTB, tag="ones")
        nc.vector.memset(ones_row, 1.0)

        # ---- DMA queues: sync/gpsimd/scalar can issue DMA; scalar only
        # used for prologue loads (ACT is exp-bound in steady state) ----
        QS = [nc.sync, nc.gpsimd, nc.scalar]
        qi = 0

        def dq():
            nonlocal qi
            qi += 1
            return QS[qi % len(QS)]

        # identity for the small v transposes
        make_identity(nc, ident)
        if LNRECIP:
            # touch Ln before any Exp so walrus loads the activation table
            # set that contains BOTH (natural_log_exp_and_others) exactly once
            dumm = consts.tile([1, 8], DT, tag="dumm")
            nc.vector.memset(dumm, 1.0)
            nc.scalar.activation(dumm, dumm, AF.Ln)

        # DMA order = first-needed first: kv weights + first x block feed
        # proj_kv(0); then q/bias weights; wo + the x tail follow.
        for kc in range(KC):
            dq().dma_start(wkv_sb[kc], wkvd[kc * P:(kc + 1) * P, :])
            dq().dma_start(xT[kc][:, 0:512], xtd[kc * P:(kc + 1) * P, 0:512])
        dq().dma_start(bkv_sb, bkvd[:, :])
        for kc in range(KC):
            dq().dma_start(wq_sb[kc], wqd[kc * P:(kc + 1) * P, :])
        for mc in range(2):
            dq().dma_start(bq_sb[:, mc:mc + 1], bqd[mc * P:(mc + 1) * P, :])
        for c in range(2):
            dq().dma_start(wo_sb[c], wod[c * P:(c + 1) * P, :])
        for kc in range(KC):
            dq().dma_start(xT[kc][:, 512:S], xtd[kc * P:(kc + 1) * P, 512:S])
        nc.vector.memset(vA[:, :, HD:HD + 1], 1.0)

        def proj_kv(nb):
            sl = slice(nb * 512, (nb + 1) * 512)
            # k|v fused projection
            ps2 = psP.tile([P, 512], DT, tag="pp", name="ps2")
            for kc in range(KC):
                nc.tensor.matmul(ps2, wkv_sb[kc], xT[kc][:, sl],
                                 start=(kc == 0), stop=(kc == KC - 1))
            nc.vector.tensor_scalar_add(kT[0:HD, sl], ps2[0:HD, :], bkv_sb[0:HD, :])
            nc.sync.dma_start(kT[HD:P, sl], kT[0:HD, sl])
            vt = vtp.tile([HD, 512], DTB, tag="vt", name="vt")
            nc.vector.tensor_scalar_add(
                vt, ps2[HD:2 * HD, :], bkv_sb[HD:P, :])
            return vt

        def proj_vtrans(nb, vt):
            for j in range(4):
                kt_i = nb * 4 + j
                vps = psP.tile([P, HD], DTB, tag="pp", name="vps")
                nc.tensor.transpose(vps, vt[:, j * P:(j + 1) * P], ident)
                nc.vector.tensor_copy(vA[:, kt_i, 0:HD], vps)

        def proj_q(nb, mc):
            sl = slice(nb * 512, (nb + 1) * 512)
            ps = psP.tile([P, 512], DT, tag="pp", name="psq")
            for kc in range(KC):
                nc.tensor.matmul(
                    ps, wq_sb[kc][:, mc * P:(mc + 1) * P], xT[kc][:, sl],
                    start=(kc == 0), stop=(kc == KC - 1))
            nc.vector.tensor_scalar(
                out=qT[mc][:, sl], in0=ps,
                scalar1=0.125, scalar2=bq_sb[:, mc:mc + 1],
                op0=mybir.AluOpType.mult, op1=mybir.AluOpType.add)
            if not ROW_TILED:
                nc.vector.tensor_copy(qTlo[mc][:, sl], qT[mc][HD:P, sl])

        def proj_pieces(nb):
            state = {}

            def p0():
                state["vt"] = proj_kv(nb)

            return [p0,
                    lambda: proj_vtrans(nb, state["vt"]),
                    lambda: proj_q(nb, 0),
                    lambda: proj_q(nb, 1)]

        def attn_pair_pipe(qb, pr, fill_iter=None):
            """Software-pipelined attention for head pair pr: scores/exp for
            kt are emitted one step ahead of AV(kt-1), so the PE never
            head-of-line blocks on the exp of the tile it is about to
            consume. Scores run row-tile-concurrent (T0/T8); AV is full-K."""
            qsl0 = qb * 512
            nkt = 4 * (qb + 1)
            acc = [psA.tile([HD + 1, 512], DT, tag="acc", name="acc") for _ in range(2)]

            def emit_scores_exp(kt):
                di = kt - 4 * qb
                off = di * P if di > 0 else 0
                wv_ = 512 - off
                ktsl = slice(kt * P, (kt + 1) * P)
                qssl = slice(qsl0 + off, qsl0 + 512)
                s0 = psS.tile([P, 512], DT, tag="ss", name="s0")
                s1 = psS.tile([P, 512], DT, tag="ss", name="s1")
                nc.tensor.matmul(s0[:, off:512], kT[0:HD, ktsl], qT[pr][0:HD, qssl],
                                 start=True, stop=True)
                nc.tensor.matmul(s1[:, off:512], kT[HD:P, ktsl], qT[pr][HD:P, qssl],
                                 start=True, stop=True)
                et0 = etp.tile([P, 512], DTB, tag="et", name="et0")
                et1 = etp.tile([P, 512], DTB, tag="et", name="et1")
                nc.scalar.activation(et0[:, off:512], s0[:, off:512], AF.Exp)
                nc.scalar.activation(et1[:, off:512], s1[:, off:512], AF.Exp)
                if di >= 0:
                    base = qb * 512 + off - kt * P
                    nc.gpsimd.affine_select(
                        out=et0[:, off:512], in_=et0[:, off:512],
                        pattern=[[1, wv_]],
                        compare_op=mybir.AluOpType.is_ge, fill=0.0,
                        base=base, channel_multiplier=-1)
                    nc.gpsimd.affine_select(
                        out=et1[:, off:512], in_=et1[:, off:512],
                        pattern=[[1, wv_]],
                        compare_op=mybir.AluOpType.is_ge, fill=0.0,
                        base=base, channel_multiplier=-1)
                return off, et0, et1

            def emit_av(kt, off, et0, et1):
                st = (kt == 0)
                sp = (kt == nkt - 1)
                o5 = slice(off, 512)
                nc.tensor.matmul(acc[0][:, o5], vA[:, kt, :], et0[:, o5], start=st, stop=sp)
                nc.tensor.matmul(acc[1][:, o5], vA[:, kt, :], et1[:, o5], start=st, stop=sp)

            prev = None
            for kt in range(nkt):
                off, et0, et1 = emit_scores_exp(kt)
                if prev is not None:
                    emit_av(*prev)
                    if fill_iter is not None and kt % 2 == 0:
                        f = next(fill_iter, None)
                        if f is not None:
                            f()
                prev = (kt, off, et0, et1)
            emit_av(*prev)

            qsl = slice(qsl0, qsl0 + 512)
            recb2 = dnp.tile([1, 1024], DTB, tag="recb2", name="recb2")
            if LNRECIP:
                # 1/den = exp(-ln(den)) on ACT, both heads batched [1,1024];
                # ln+exp share one activation table set -> no table thrash.
                dens2 = dnp.tile([HD + 1, 1024], DT, tag="dens2", name="dens2")
                for hh in range(2):
                    nc.vector.tensor_copy(
                        dens2[HD:HD + 1, hh * 512:(hh + 1) * 512],
                        acc[hh][HD:HD + 1, :])
                lnt = dnp.tile([HD + 1, 1024], DT, tag="lnt", name="lnt")
                nc.scalar.activation(
                    lnt[HD:HD + 1, :], dens2[HD:HD + 1, :], AF.Ln)
                rec2 = dnp.tile([HD + 1, 1024], DT, tag="rec2", name="rec2")
                nc.scalar.activation(
                    rec2[HD:HD + 1, :], lnt[HD:HD + 1, :], AF.Exp, scale=-1.0)
                nc.vector.tensor_copy(recb2, rec2[HD:HD + 1, :])
            else:
                for hh in range(2):
                    dens = dnp.tile([HD + 1, 512], DT, tag="dens", name="dens")
                    nc.vector.tensor_copy(dens[HD:HD + 1, :], acc[hh][HD:HD + 1, :])
                    rec = dnp.tile([1, 512], DT, tag="rec", name="rec")
                    nc.vector.reciprocal(rec, dens[HD:HD + 1, :])
                    nc.vector.tensor_copy(
                        recb2[0:1, hh * 512:(hh + 1) * 512], rec)
            for hh in range(2):
                h = 2 * pr + hh
                c, r0 = h // 2, (h % 2) * HD
                recB = rbp.tile([P, 512], DTB, tag="recB", name="recB")
                rbps = psP.tile([HD, 512], DT, tag="pp", name="rbps")
                nc.tensor.matmul(
                    rbps, ones_row, recb2[0:1, hh * 512:(hh + 1) * 512],
                    start=True, stop=True)
                nc.vector.tensor_copy(recB[r0:r0 + HD, :], rbps)
                nc.vector.tensor_mul(
                    oT[c][r0:r0 + HD, qsl], acc[hh][0:HD, :], recB[r0:r0 + HD, :])

        def attn_pair(qb, pr, fill_iter=None):
            """Attention for head pair pr (local heads 2pr, 2pr+1), q block qb."""
            qsl0 = qb * 512
            nkt = 4 * (qb + 1)
            accA = [psA.tile([HD + 1, 512], DT, tag="acc", name="accA") for _ in range(2)]
            accB = (
                [psA.tile([HD + 1, 512], DT, tag="acc", name="accB") for _ in range(2)]
                if ROW_TILED else [None, None])
            for kt in range(nkt):
                di = kt - 4 * qb
                off = di * P if di > 0 else 0
                wv_ = 512 - off
                ktsl = slice(kt * P, (kt + 1) * P)
                qssl = slice(qsl0 + off, qsl0 + 512)
                s0 = psS.tile([P, 512], DT, tag="ss", name="s0")
                s1 = psS.tile([P, 512], DT, tag="ss", name="s1")
                nc.tensor.matmul(s0[:, off:512], kT[0:HD, ktsl], qT[pr][0:HD, qssl],
                                 start=True, stop=True)
                if ROW_TILED:
                    nc.tensor.matmul(s1[:, off:512], kT[HD:P, ktsl], qT[pr][HD:P, qssl],
                                     start=True, stop=True)
                else:
                    nc.tensor.matmul(s1[:, off:512], kT[0:HD, ktsl], qTlo[pr][:, qssl],
                                     start=True, stop=True)
                et0 = etp.tile([P, 512], DTB, tag="et", name="et0")
                et1 = etp.tile([P, 512], DTB, tag="et", name="et1")
                nc.scalar.activation(et0[:, off:512], s0[:, off:512], AF.Exp)
                nc.scalar.activation(et1[:, off:512], s1[:, off:512], AF.Exp)
                if di >= 0:
                    # causal: keep where (qb*512 + off + f) - p - kt*128 >= 0,
                    # i.e. q_abs >= key_abs; the AP starts at column `off` so
                    # fold it into base.
                    base = qb * 512 + off - kt * P
                    nc.gpsimd.affine_select(
                        out=et0[:, off:512], in_=et0[:, off:512],
                        pattern=[[1, wv_]],
                        compare_op=mybir.AluOpType.is_ge, fill=0.0,
                        base=base, channel_multiplier=-1)
                    nc.gpsimd.affine_select(
                        out=et1[:, off:512], in_=et1[:, off:512],
                        pattern=[[1, wv_]],
                        compare_op=mybir.AluOpType.is_ge, fill=0.0,
                        base=base, channel_multiplier=-1)
                if fill_iter is not None and kt % 2 == 1:
                    # interleave a projection/output-proj piece: the PE chews
                    # on it while ACT runs the exps, instead of idling.
                    f = next(fill_iter, None)
                    if f is not None:
                        f()
                st = (kt == 0)
                sp = (kt == nkt - 1)
                o5 = slice(off, 512)
                if ROW_TILED:
                    # split-K AV: T0 half (keys 0-63) and T8 half (keys
                    # 64-127) run concurrently; consecutive instructions
                    # alternate banks.
                    nc.tensor.matmul(accA[0][:, o5], vA[0:HD, kt, :], et0[0:HD, o5], start=st, stop=sp)
                    nc.tensor.matmul(accB[1][:, o5], vA[HD:P, kt, :], et1[HD:P, o5], start=st, stop=sp)
                    nc.tensor.matmul(accB[0][:, o5], vA[HD:P, kt, :], et0[HD:P, o5], start=st, stop=sp)
                    nc.tensor.matmul(accA[1][:, o5], vA[0:HD, kt, :], et1[0:HD, o5], start=st, stop=sp)
                else:
                    nc.tensor.matmul(accA[0][:, o5], vA[:, kt, :], et0[:, o5], start=st, stop=sp)
                    nc.tensor.matmul(accA[1][:, o5], vA[:, kt, :], et1[:, o5], start=st, stop=sp)
            qsl = slice(qsl0, qsl0 + 512)
            for hh in range(2):
                h = 2 * pr + hh
                c, r0 = h // 2, (h % 2) * HD
                dens = dnp.tile([HD + 1, 512], DT, tag="dens", name="dens")
                if ROW_TILED:
                    # only one PSUM operand per DVE op: stage accB in SBUF
                    obf = dnp.tile([HD + 1, 512], DT, tag="obf", name="obf")
                    nc.vector.tensor_copy(obf, accB[hh])
                    nc.vector.tensor_add(
                        dens[HD:HD + 1, :], accA[hh][HD:HD + 1, :], obf[HD:HD + 1, :])
                else:
                    nc.vector.tensor_copy(dens[HD:HD + 1, :], accA[hh][HD:HD + 1, :])
                recb = dnp.tile([1, 512], DTB, tag="recb", name="recb")
                if APPROX_RECIP:
                    # custom-DVE op: keep in/out partition-aligned (row 64)
                    rec = dnp.tile([HD + 1, 512], DT, tag="rec", name="rec")
                    nc.vector.reciprocal_approx_fast(
                        out=rec[HD:HD + 1, :], in_=dens[HD:HD + 1, :])
                    nc.vector.tensor_copy(recb, rec[HD:HD + 1, :])
                else:
                    rec = dnp.tile([1, 512], DT, tag="rec", name="rec")
                    nc.vector.reciprocal(rec, dens[HD:HD + 1, :])
                    nc.vector.tensor_copy(recb, rec)
                recB = rbp.tile([P, 512], DTB, tag="recB", name="recB")
                if PBCAST:
                    nc.gpsimd.partition_broadcast(
                        recB[r0:r0 + HD, :], recb, channels=HD)
                else:
                    # broadcast 1/den across 64 head dims via a K=1 matmul;
                    # psP (not psS) so the next pair's scores aren't blocked
                    rbps = psP.tile([HD, 512], DT, tag="pp", name="rbps")
                    nc.tensor.matmul(rbps, ones_row, recb, start=True, stop=True)
                    nc.vector.tensor_copy(recB[r0:r0 + HD, :], rbps)
                if ROW_TILED:
                    nc.vector.tensor_add(
                        oT[c][r0:r0 + HD, qsl], accA[hh][0:HD, :], obf[0:HD, :])
                    nc.vector.tensor_mul(
                        oT[c][r0:r0 + HD, qsl], oT[c][r0:r0 + HD, qsl], recB[r0:r0 + HD, :])
                else:
                    nc.vector.tensor_mul(
                        oT[c][r0:r0 + HD, qsl], accA[hh][0:HD, :], recB[r0:r0 + HD, :])

        def wo_st(st):
            yt = ysb.tile([P, D], DT, tag="y", name="yt")
            for nb2 in range(2):
                yps = psP.tile([P, 512], DT, tag="pp", name="yps")
                for c in range(2):
                    nc.tensor.matmul(
                        yps, oT[c][:, st * P:(st + 1) * P],
                        wo_sb[c][:, nb2 * 512:(nb2 + 1) * 512],
                        start=(c == 0), stop=(c == 1))
                nc.vector.tensor_copy(yt[:, nb2 * 512:(nb2 + 1) * 512], yps)
            QS[st % 2].dma_start(out_p[st * P:(st + 1) * P, :], yt)

        def wo_pieces(bl):
            return [(lambda st=st: wo_st(st)) for st in range(bl * 4, bl * 4 + 4)]

        attn = attn_pair_pipe if PIPE else attn_pair

        if FILL:
            for f in proj_pieces(0):
                f()
            for qb in range(4):
                fillers = []
                if qb + 1 < 4:
                    fillers += proj_pieces(qb + 1)
                if qb >= 1:
                    fillers += wo_pieces(qb - 1)
                fill_iter = iter(fillers)
                attn(qb, 0, fill_iter)
                attn(qb, 1, fill_iter)
                for f in fill_iter:
                    f()
            for f in wo_pieces(3):
                f()
        else:
            for nb in range(4):
                for f in proj_pieces(nb):
                    f()
                if nb >= 1:
                    for f in wo_pieces(nb - 1):
                        f()
                attn(nb, 0)
                attn(nb, 1)
            for f in wo_pieces(3):
                f()

    nc.compile()
    return nc


def prepare_in_maps(x, wq, bq, wk, bk, wv, bv, wo):
    x = np.asarray(x, dtype=np.float32)
    xb = [np.ascontiguousarray(x[b].T).astype(BF16) for b in range(B)]
    wqb = np.asarray(wq, dtype=np.float32).astype(BF16)
    wkb = np.asarray(wk, dtype=np.float32).astype(BF16)
    wvb = np.asarray(wv, dtype=np.float32).astype(BF16)
    wob = np.asarray(wo, dtype=np.float32).astype(BF16)
    bq = np.asarray(bq, dtype=np.float32)
    bk = np.asarray(bk, dtype=np.float32)
    bv = np.asarray(bv, dtype=np.float32)

    in_maps = []
    for c in range(N_CORES):
        b, g = c // 4, c % 4
        sq = slice(g * DG, (g + 1) * DG)
        sk = slice(g * HD, (g + 1) * HD)
        in_maps.append({
            "xtd": xb[b],
            "wqd": np.ascontiguousarray(wqb[:, sq]),
            "wkvd": np.ascontiguousarray(
                np.concatenate([wkb[:, sk], wvb[:, sk]], axis=1)),
            "wod": np.ascontiguousarray(wob[sq, :]),
            "bqd": np.ascontiguousarray((bq[sq] * 0.125).reshape(DG, 1)),
            "bkvd": np.ascontiguousarray(
                np.concatenate([bk[sk], bv[sk]]).reshape(2 * HD, 1)),
        })
    return in_maps


def kernel(x, mask, wq, bq, wk, bk, wv, bv, wo, bo):
    bo = np.asarray(bo, dtype=np.float32)
    in_maps = prepare_in_maps(x, wq, bq, wk, bk, wv, bv, wo)
    results = _run(in_maps)

    out = np.empty((B, S, D), dtype=np.float32)
    for b in range(B):
        acc = results[b * 4 + 0]["out_p"].astype(np.float64)
        for g in range(1, 4):
            acc += results[b * 4 + g]["out_p"]
        out[b] = (acc + bo).astype(np.float32)
    return out


def _get_runner():
    """Build (once) a jitted shard_map callable executing the compiled
    kernel on 8 cores. Adapted from concourse.bass2jax.run_bass_via_pjrt,
    minus output-buffer donation so the callable is re-invokable for
    timing."""
    if "runner" in _CACHE:
        return _CACHE["runner"]
    import jax
    from jax.experimental.shard_map import shard_map
    from jax.sharding import Mesh, PartitionSpec
    from concourse import bass2jax
    from concourse.bass2jax import _bass_exec_p, install_neuronx_cc_hook

    install_neuronx_cc_hook()
    nc = build_nc()
    partition_name = (
        nc.partition_id_tensor.name if nc.partition_id_tensor else None
    )

    in_names, out_names, out_avals, zero_outs = [], [], [], []
    for alloc in nc.m.functions[0].allocations:
        if not isinstance(alloc, mybir.MemoryLocationSet):
            continue
        name = alloc.memorylocations[0].name
        if alloc.kind == "ExternalInput":
            if name != partition_name:
                in_names.append(name)
        elif alloc.kind == "ExternalOutput":
            out_names.append(name)
            shape = tuple(alloc.tensor_shape)
            dtype = mybir.dt.np(alloc.dtype)
            out_avals.append(jax.core.ShapedArray(shape, dtype))
            zero_outs.append(np.zeros(shape, dtype))
    n_params = len(in_names)
    all_names = in_names + out_names
    if partition_name is not None:
        all_names = all_names + [partition_name]

    def _body(*args):
        operands = list(args)
        if partition_name is not None:
            operands.append(bass2jax.partition_id_tensor())
        outs = _bass_exec_p.bind(
            *operands,
            out_avals=tuple(out_avals),
            in_names=tuple(all_names),
            out_names=tuple(out_names),
            lowering_input_output_aliases=(),
            sim_require_finite=True,
            sim_require_nnan=True,
            nc=nc,
        )
        return tuple(outs)

    devices = jax.devices()[:N_CORES]
    mesh = Mesh(np.asarray(devices), ("core",))
    n_all = n_params + len(out_names)
    sharded = jax.jit(
        shard_map(
            _body,
            mesh=mesh,
            in_specs=(PartitionSpec("core"),) * n_all,
            out_specs=(PartitionSpec("core"),) * len(out_names),
            check_rep=False,
        ),
        keep_unused=True,
    )
    runner = {
        "sharded": sharded,
        "in_names": in_names,
        "out_names": out_names,
        "out_avals": out_avals,
        "zero_outs": zero_outs,
        "mesh": mesh,
        "nc": nc,
    }
    _CACHE["runner"] = runner
    return runner


def _run(in_maps):
    r = _get_runner()
    concat_in = [
        np.concatenate([np.asarray(in_maps[c][n]) for c in range(N_CORES)], axis=0)
        for n in r["in_names"]
    ]
    concat_zeros = [
        np.zeros((N_CORES * z.shape[0], *z.shape[1:]), z.dtype)
        for z in r["zero_outs"]
    ]
    out_arrs = r["sharded"](*concat_in, *concat_zeros)
    _CACHE["last_args"] = (concat_in, concat_zeros)
    return [
        {
            n: np.asarray(out_arrs[i]).reshape(
                N_CORES, *r["out_avals"][i].shape
            )[c]
            for i, n in enumerate(r["out_names"])
        }
        for c in range(N_CORES)
    ]


def bench(iters=10):
    """Re-execute the last-run kernel with device-resident inputs and
    return per-call wall times (s). Outputs stay on device."""
    import time as _time
    import jax
    from jax.sharding import NamedSharding, PartitionSpec

    r = _CACHE["runner"]
    concat_in, concat_zeros = _CACHE["last_args"]
    sh = NamedSharding(r["mesh"], PartitionSpec("core"))
    dev_args = [jax.device_put(a, sh) for a in (*concat_in, *concat_zeros)]
    for a in dev_args:
        a.block_until_ready()
    times = []
    for _ in range(iters):
        t0 = _time.perf_counter()
        outs = r["sharded"](*dev_args)
        for o in outs:
            o.block_until_ready()
        times.append(_time.perf_counter() - t0)
    return times


# revision 44
# speedup vs baseline: 411.9389x; 1.0687x over previous
"""GQA attention kernel for 8 Trainium2 NeuronCores.

Problem: B=2, S=2048, D=1024, 16 Q heads / 4 KV heads (GQA), causal,
y = softmax((x@wq+bq)(x@wk+bk)^T / 8, causal) @ (x@wv+bv) @ wo + bo

Sharding: core c -> (batch b = c//4, kv-group g = c%4). Each core computes
its batch's attention for 4 Q heads (= 1 KV head) and the partial output
projection through wo[g*256:(g+1)*256, :]. Host sums the 4 partials per
batch and adds bo.

Device kernel layout (v2):
  - x arrives pre-transposed from host as xT [D, S] bf16: no PE transposes.
  - kT [128, S]: rows 0-63 = k^T, rows 64-127 = duplicate (SBUF->SBUF DMA),
    so the PE can run in 64x128 row-tiled mode with two concurrent tiles
    (T0 = SBUF partitions 0-63, T8 = partitions 64-127).
  - qT per head-pair [128, S]: rows 0-63 = even head, 64-127 = odd head.
  - scores for the two heads of a pair run CONCURRENTLY on T0/T8.
  - AV contraction (128 keys) is split into two 64-key halves on T0/T8
    accumulating into separate PSUM banks (accA/accB); summed during the
    softmax-normalize step on DVE. No PE mode switches inside attention.
  - softmax denominator comes from a ones-column appended to v; 1/den via
    DVE reciprocal_approx_fast, partition-broadcast on gpsimd, applied on
    DVE. exp() width-trimmed on causal-diagonal tiles.
"""

import sys
from contextlib import ExitStack

import numpy as np
import ml_dtypes

if "/opt/trn_rl_repo" not in sys.path:
    sys.path.insert(0, "/opt/trn_rl_repo")

import concourse.bass as bass
import concourse.tile as tile
from concourse import bacc, mybir
from concourse.masks import make_identity

B, S, D = 2, 2048, 1024
H, KVH, HD = 16, 4, 64
GQ = H // KVH        # 4 q heads per core
DG = GQ * HD         # 256 q dims per core
P = 128
KC = D // P          # 8 contraction chunks over D
NKT = S // P         # 16 key tiles
NQB = S // 512       # 4 query blocks
N_CORES = 8

DT = mybir.dt.float32
DTB = mybir.dt.bfloat16
AF = mybir.ActivationFunctionType
BF16 = ml_dtypes.bfloat16

_CACHE = {}

import os
# bisect flags (HW debug)
ROW_TILED = os.environ.get("K_ROW_TILED", "1") == "1"
# reciprocal_approx_fast (custom DVE ucode) and gpsimd partition_broadcast
# both produce garbage on HW through this execution path (sim is fine) —
# keep them off; plain DVE reciprocal + ones-row matmul broadcast work.
APPROX_RECIP = os.environ.get("K_APPROX_RECIP", "0") == "1"
PBCAST = os.environ.get("K_PBCAST", "0") == "1"
FILL = os.environ.get("K_FILL", "1") == "1"
# software-pipelined kt loop: scores run one step ahead of AV so the exp
# latency is covered by useful PE work; AV is full-K (single acc per head)
# freeing two PSUM banks for the deeper scores pipeline.
PIPE = os.environ.get("K_PIPE", "1") == "1"
# 1/den via exp(-ln(den)) on ACT: ln+exp live in the same activation table
# set (no table thrash), batched per pair as [1,1024] — replaces the 3.4us
# per-head DVE reciprocal.
LNRECIP = os.environ.get("K_LNRECIP", "1") == "1"


def build_nc():
    nc = bacc.Bacc(
        "TRN2",
        target_bir_lowering=False,
        debug=False,
        enable_asserts=False,
        num_devices=N_CORES,
    )
    xtd = nc.dram_tensor("xtd", [D, S], DTB, kind="ExternalInput").ap()
    wqd = nc.dram_tensor("wqd", [D, DG], DTB, kind="ExternalInput").ap()
    wkvd = nc.dram_tensor("wkvd", [D, 2 * HD], DTB, kind="ExternalInput").ap()
    wod = nc.dram_tensor("wod", [DG, D], DTB, kind="ExternalInput").ap()
    bqd = nc.dram_tensor("bqd", [DG, 1], DT, kind="ExternalInput").ap()
    bkvd = nc.dram_tensor("bkvd", [2 * HD, 1], DT, kind="ExternalInput").ap()
    out_p = nc.dram_tensor("out_p", [S, D], DT, kind="ExternalOutput").ap()

    with tile.TileContext(nc) as tc, ExitStack() as ctx:
        consts = ctx.enter_context(tc.tile_pool(name="consts", bufs=1))
        etp = ctx.enter_context(tc.tile_pool(name="etp", bufs=6))
        vtp = ctx.enter_context(tc.tile_pool(name="vtp", bufs=2))
        dnp = ctx.enter_context(tc.tile_pool(name="dnp", bufs=3))
        rbp = ctx.enter_context(tc.tile_pool(name="rbp", bufs=3))
        ysb = ctx.enter_context(tc.tile_pool(name="ysb", bufs=3))
        psP = ctx.enter_context(tc.tile_pool(name="psP", bufs=2, space="PSUM"))
        psS = ctx.enter_context(
            tc.tile_pool(name="psS", bufs=(4 if PIPE else 2), space="PSUM"))
        psA = ctx.enter_context(
            tc.tile_pool(name="psA", bufs=(2 if PIPE else 4), space="PSUM"))

        # persistent SBUF
        xT = [consts.tile([P, S], DTB, tag=f"xT{kc}", name=f"xT{kc}") for kc in range(KC)]
        kT = consts.tile([P, S], DTB, tag="kT")
        qT = [consts.tile([P, S], DTB, tag=f"qT{pr}", name=f"qT{pr}") for pr in range(2)]
        qTlo = (None if ROW_TILED else
                [consts.tile([HD, S], DTB, tag=f"qTlo{pr}", name=f"qTlo{pr}")
                 for pr in range(2)])
        vA = consts.tile([P, NKT, HD + 1], DTB, tag="vA")
        oT = [consts.tile([P, S], DTB, tag=f"oT{c}", name=f"oT{c}") for c in range(2)]
        wq_sb = [consts.tile([P, DG], DTB, tag=f"wq{kc}", name=f"wq{kc}") for kc in range(KC)]
        wkv_sb = [consts.tile([P, 2 * HD], DTB, tag=f"wkv{kc}", name=f"wkv{kc}") for kc in range(KC)]
        wo_sb = [consts.tile([P, D], DTB, tag=f"wo{c}", name=f"wo{c}") for c in range(2)]
        bq_sb = consts.tile([P, 2], DT, tag="bq")
        # rows 0-63 = bk (aligned with k's PSUM rows), 64-127 = bv
        bkv_sb = consts.tile([P, 1], DT, tag="bkv")
        ident = consts.tile([HD, HD], DTB, tag="ident")
        ones_row = consts.tile([1, HD], DTB, tag="ones")
        nc.vector.memset(ones_row, 1.0)

        # ---- DMA queues: sync/gpsimd/scalar can issue DMA; scalar only
        # used for prologue loads (ACT is exp-bound in steady state) ----
        QS = [nc.sync, nc.gpsimd, nc.scalar]
        qi = 0

        def dq():
            nonlocal qi
            qi += 1
            return QS[qi % len(QS)]

        # identity for the small v transposes
        make_identity(nc, ident)
        # DMA order = first-needed first: kv weights + first x block feed
        # proj_kv(0); then q/bias weights; wo + the x tail follow.
        for kc in range(KC):
            dq().dma_start(wkv_sb[kc], wkvd[kc * P:(kc + 1) * P, :])
            dq().dma_start(xT[kc][:, 0:512], xtd[kc * P:(kc + 1) * P, 0:512])
        dq().dma_start(bkv_sb, bkvd[:, :])
        for kc in range(KC):
            dq().dma_start(wq_sb[kc], wqd[kc * P:(kc + 1) * P, :])
        for mc in range(2):
            dq().dma_start(bq_sb[:, mc:mc + 1], bqd[mc * P:(mc + 1) * P, :])
        for c in range(2):
            dq().dma_start(wo_sb[c], wod[c * P:(c + 1) * P, :])
        for kc in range(KC):
            dq().dma_start(xT[kc][:, 512:S], xtd[kc * P:(kc + 1) * P, 512:S])
        nc.vector.memset(vA[:, :, HD:HD + 1], 1.0)

        def proj_kv(nb):
            sl = slice(nb * 512, (nb + 1) * 512)
            # k|v fused projection
            ps2 = psP.tile([P, 512], DT, tag="pp", name="ps2")
            for kc in range(KC):
                nc.tensor.matmul(ps2, wkv_sb[kc], xT[kc][:, sl],
                                 start=(kc == 0), stop=(kc == KC - 1))
            nc.vector.tensor_scalar_add(kT[0:HD, sl], ps2[0:HD, :], bkv_sb[0:HD, :])
            nc.sync.dma_start(kT[HD:P, sl], kT[0:HD, sl])
            vt = vtp.tile([HD, 512], DTB, tag="vt", name="vt")
            nc.vector.tensor_scalar_add(
                vt, ps2[HD:2 * HD, :], bkv_sb[HD:P, :])
            return vt

        def proj_vtrans(nb, vt):
            for j in range(4):
                kt_i = nb * 4 + j
                vps = psP.tile([P, HD], DTB, tag="pp", name="vps")
                nc.tensor.transpose(vps, vt[:, j * P:(j + 1) * P], ident)
                nc.vector.tensor_copy(vA[:, kt_i, 0:HD], vps)

        def proj_q(nb, mc):
            sl = slice(nb * 512, (nb + 1) * 512)
            ps = psP.tile([P, 512], DT, tag="pp", name="psq")
            for kc in range(KC):
                nc.tensor.matmul(
                    ps, wq_sb[kc][:, mc * P:(mc + 1) * P], xT[kc][:, sl],
                    start=(kc == 0), stop=(kc == KC - 1))
            nc.vector.tensor_scalar(
                out=qT[mc][:, sl], in0=ps,
                scalar1=0.125, scalar2=bq_sb[:, mc:mc + 1],
                op0=mybir.AluOpType.mult, op1=mybir.AluOpType.add)
            if not ROW_TILED:
                nc.vector.tensor_copy(qTlo[mc][:, sl], qT[mc][HD:P, sl])

        def proj_pieces(nb):
            state = {}

            def p0():
                state["vt"] = proj_kv(nb)

            return [p0,
                    lambda: proj_vtrans(nb, state["vt"]),
                    lambda: proj_q(nb, 0),
                    lambda: proj_q(nb, 1)]

        def attn_pair_pipe(qb, pr, fill_iter=None):
            """Software-pipelined attention for head pair pr: scores/exp for
            kt are emitted one step ahead of AV(kt-1), so the PE never
            head-of-line blocks on the exp of the tile it is about to
            consume. Scores run row-tile-concurrent (T0/T8); AV is full-K."""
            qsl0 = qb * 512
            nkt = 4 * (qb + 1)
            acc = [psA.tile([HD + 1, 512], DT, tag="acc", name="acc") for _ in range(2)]

            def emit_scores_exp(kt):
                di = kt - 4 * qb
                off = di * P if di > 0 else 0
                wv_ = 512 - off
                ktsl = slice(kt * P, (kt + 1) * P)
                qssl = slice(qsl0 + off, qsl0 + 512)
                s0 = psS.tile([P, 512], DT, tag="ss", name="s0")
                s1 = psS.tile([P, 512], DT, tag="ss", name="s1")
                nc.tensor.matmul(s0[:, off:512], kT[0:HD, ktsl], qT[pr][0:HD, qssl],
                                 start=True, stop=True)
                nc.tensor.matmul(s1[:, off:512], kT[HD:P, ktsl], qT[pr][HD:P, qssl],
                                 start=True, stop=True)
                et0 = etp.tile([P, 512], DTB, tag="et", name="et0")
                et1 = etp.tile([P, 512], DTB, tag="et", name="et1")
                nc.scalar.activation(et0[:, off:512], s0[:, off:512], AF.Exp)
                nc.scalar.activation(et1[:, off:512], s1[:, off:512], AF.Exp)
                if di >= 0:
                    base = qb * 512 + off - kt * P
                    nc.gpsimd.affine_select(
                        out=et0[:, off:512], in_=et0[:, off:512],
                        pattern=[[1, wv_]],
                        compare_op=mybir.AluOpType.is_ge, fill=0.0,
                        base=base, channel_multiplier=-1)
                    nc.gpsimd.affine_select(
                        out=et1[:, off:512], in_=et1[:, off:512],
                        pattern=[[1, wv_]],
                        compare_op=mybir.AluOpType.is_ge, fill=0.0,
                        base=base, channel_multiplier=-1)
                return off, et0, et1

            def emit_av(kt, off, et0, et1):
                st = (kt == 0)
                sp = (kt == nkt - 1)
                o5 = slice(off, 512)
                nc.tensor.matmul(acc[0][:, o5], vA[:, kt, :], et0[:, o5], start=st, stop=sp)
                nc.tensor.matmul(acc[1][:, o5], vA[:, kt, :], et1[:, o5], start=st, stop=sp)

            prev = None
            for kt in range(nkt):
                off, et0, et1 = emit_scores_exp(kt)
                if prev is not None:
                    emit_av(*prev)
                    if fill_iter is not None and kt % 2 == 0:
                        f = next(fill_iter, None)
                        if f is not None:
                            f()
                prev = (kt, off, et0, et1)
            emit_av(*prev)

            qsl = slice(qsl0, qsl0 + 512)
            recb2 = dnp.tile([1, 1024], DTB, tag="recb2", name="recb2")
            if LNRECIP:
                # both heads' denominators in one tile: h0 at row 64
                # (partition-aligned copy), h1 at row 0 (the proven 64->0
                # remap); ONE [65,512] DVE reciprocal covers both rows
                # (rows 1-63 are garbage and unused).
                dens4 = dnp.tile([HD + 1, 512], DT, tag="dens4", name="dens4")
                nc.vector.tensor_copy(dens4[HD:HD + 1, :], acc[0][HD:HD + 1, :])
                nc.vector.tensor_copy(dens4[0:1, :], acc[1][HD:HD + 1, :])
                rec4 = dnp.tile([HD + 1, 512], DT, tag="rec4", name="rec4")
                nc.vector.reciprocal(rec4, dens4)
                nc.vector.tensor_copy(recb2[0:1, 0:512], rec4[HD:HD + 1, :])
                nc.vector.tensor_copy(recb2[0:1, 512:1024], rec4[0:1, :])
            else:
                for hh in range(2):
                    dens = dnp.tile([HD + 1, 512], DT, tag="dens", name="dens")
                    nc.vector.tensor_copy(dens[HD:HD + 1, :], acc[hh][HD:HD + 1, :])
                    rec = dnp.tile([1, 512], DT, tag="rec", name="rec")
                    nc.vector.reciprocal(rec, dens[HD:HD + 1, :])
                    nc.vector.tensor_copy(
                        recb2[0:1, hh * 512:(hh + 1) * 512], rec)
            for hh in range(2):
                h = 2 * pr + hh
                c, r0 = h // 2, (h % 2) * HD
                recB = rbp.tile([P, 512], DTB, tag="recB", name="recB")
                rbps = psP.tile([HD, 512], DT, tag="pp", name="rbps")
                nc.tensor.matmul(
                    rbps, ones_row, recb2[0:1, hh * 512:(hh + 1) * 512],
                    start=True, stop=True)
                nc.vector.tensor_copy(recB[r0:r0 + HD, :], rbps)
                nc.vector.tensor_mul(
                    oT[c][r0:r0 + HD, qsl], acc[hh][0:HD, :], recB[r0:r0 + HD, :])

        def attn_pair(qb, pr, fill_iter=None):
            """Attention for head pair pr (local heads 2pr, 2pr+1), q block qb."""
            qsl0 = qb * 512
            nkt = 4 * (qb + 1)
            accA = [psA.tile([HD + 1, 512], DT, tag="acc", name="accA") for _ in range(2)]
            accB = (
                [psA.tile([HD + 1, 512], DT, tag="acc", name="accB") for _ in range(2)]
                if ROW_TILED else [None, None])
            for kt in range(nkt):
                di = kt - 4 * qb
                off = di * P if di > 0 else 0
                wv_ = 512 - off
                ktsl = slice(kt * P, (kt + 1) * P)
                qssl = slice(qsl0 + off, qsl0 + 512)
                s0 = psS.tile([P, 512], DT, tag="ss", name="s0")
                s1 = psS.tile([P, 512], DT, tag="ss", name="s1")
                nc.tensor.matmul(s0[:, off:512], kT[0:HD, ktsl], qT[pr][0:HD, qssl],
                                 start=True, stop=True)
                if ROW_TILED:
                    nc.tensor.matmul(s1[:, off:512], kT[HD:P, ktsl], qT[pr][HD:P, qssl],
                                     start=True, stop=True)
                else:
                    nc.tensor.matmul(s1[:, off:512], kT[0:HD, ktsl], qTlo[pr][:, qssl],
                                     start=True, stop=True)
                et0 = etp.tile([P, 512], DTB, tag="et", name="et0")
                et1 = etp.tile([P, 512], DTB, tag="et", name="et1")
                nc.scalar.activation(et0[:, off:512], s0[:, off:512], AF.Exp)
                nc.scalar.activation(et1[:, off:512], s1[:, off:512], AF.Exp)
                if di >= 0:
                    # causal: keep where (qb*512 + off + f) - p - kt*128 >= 0,
                    # i.e. q_abs >= key_abs; the AP starts at column `off` so
                    # fold it into base.
                    base = qb * 512 + off - kt * P
                    nc.gpsimd.affine_select(
                        out=et0[:, off:512], in_=et0[:, off:512],
                        pattern=[[1, wv_]],
                        compare_op=mybir.AluOpType.is_ge, fill=0.0,
                        base=base, channel_multiplier=-1)
                    nc.gpsimd.affine_select(
                        out=et1[:, off:512], in_=et1[:, off:512],
                        pattern=[[1, wv_]],
                        compare_op=mybir.AluOpType.is_ge, fill=0.0,
                        base=base, channel_multiplier=-1)
                if fill_iter is not None and kt % 2 == 1:
                    # interleave a projection/output-proj piece: the PE chews
                    # on it while ACT runs the exps, instead of idling.
                    f = next(fill_iter, None)
                    if f is not None:
                        f()
                st = (kt == 0)
                sp = (kt == nkt - 1)
                o5 = slice(off, 512)
                if ROW_TILED:
                    # split-K AV: T0 half (keys 0-63) and T8 half (keys
                    # 64-127) run concurrently; consecutive instructions
                    # alternate banks.
                    nc.tensor.matmul(accA[0][:, o5], vA[0:HD, kt, :], et0[0:HD, o5], start=st, stop=sp)
                    nc.tensor.matmul(accB[1][:, o5], vA[HD:P, kt, :], et1[HD:P, o5], start=st, stop=sp)
                    nc.tensor.matmul(accB[0][:, o5], vA[HD:P, kt, :], et0[HD:P, o5], start=st, stop=sp)
                    nc.tensor.matmul(accA[1][:, o5], vA[0:HD, kt, :], et1[0:HD, o5], start=st, stop=sp)
                else:
                    nc.tensor.matmul(accA[0][:, o5], vA[:, kt, :], et0[:, o5], start=st, stop=sp)
                    nc.tensor.matmul(accA[1][:, o5], vA[:, kt, :], et1[:, o5], start=st, stop=sp)
            qsl = slice(qsl0, qsl0 + 512)
            for hh in range(2):
                h = 2 * pr + hh
                c, r0 = h // 2, (h % 2) * HD
                dens = dnp.tile([HD + 1, 512], DT, tag="dens", name="dens")
                if ROW_TILED:
                    # only one PSUM operand per DVE op: stage accB in SBUF
                    obf = dnp.tile([HD + 1, 512], DT, tag="obf", name="obf")
                    nc.vector.tensor_copy(obf, accB[hh])
                    nc.vector.tensor_add(
                        dens[HD:HD + 1, :], accA[hh][HD:HD + 1, :], obf[HD:HD + 1, :])
                else:
                    nc.vector.tensor_copy(dens[HD:HD + 1, :], accA[hh][HD:HD + 1, :])
                recb = dnp.tile([1, 512], DTB, tag="recb", name="recb")
                if APPROX_RECIP:
                    # custom-DVE op: keep in/out partition-aligned (row 64)
                    rec = dnp.tile([HD + 1, 512], DT, tag="rec", name="rec")
                    nc.vector.reciprocal_approx_fast(
                        out=rec[HD:HD + 1, :], in_=dens[HD:HD + 1, :])
                    nc.vector.tensor_copy(recb, rec[HD:HD + 1, :])
                else:
                    rec = dnp.tile([1, 512], DT, tag="rec", name="rec")
                    nc.vector.reciprocal(rec, dens[HD:HD + 1, :])
                    nc.vector.tensor_copy(recb, rec)
                recB = rbp.tile([P, 512], DTB, tag="recB", name="recB")
                if PBCAST:
                    nc.gpsimd.partition_broadcast(
                        recB[r0:r0 + HD, :], recb, channels=HD)
                else:
                    # broadcast 1/den across 64 head dims via a K=1 matmul;
                    # psP (not psS) so the next pair's scores aren't blocked
                    rbps = psP.tile([HD, 512], DT, tag="pp", name="rbps")
                    nc.tensor.matmul(rbps, ones_row, recb, start=True, stop=True)
                    nc.vector.tensor_copy(recB[r0:r0 + HD, :], rbps)
                if ROW_TILED:
                    nc.vector.tensor_add(
                        oT[c][r0:r0 + HD, qsl], accA[hh][0:HD, :], obf[0:HD, :])
                    nc.vector.tensor_mul(
                        oT[c][r0:r0 + HD, qsl], oT[c][r0:r0 + HD, qsl], recB[r0:r0 + HD, :])
                else:
                    nc.vector.tensor_mul(
                        oT[c][r0:r0 + HD, qsl], accA[hh][0:HD, :], recB[r0:r0 + HD, :])

        def wo_st(st):
            yt = ysb.tile([P, D], DT, tag="y", name="yt")
            for nb2 in range(2):
                yps = psP.tile([P, 512], DT, tag="pp", name="yps")
                for c in range(2):
                    nc.tensor.matmul(
                        yps, oT[c][:, st * P:(st + 1) * P],
                        wo_sb[c][:, nb2 * 512:(nb2 + 1) * 512],
                        start=(c == 0), stop=(c == 1))
                nc.vector.tensor_copy(yt[:, nb2 * 512:(nb2 + 1) * 512], yps)
            QS[st % 2].dma_start(out_p[st * P:(st + 1) * P, :], yt)

        def wo_pieces(bl):
            return [(lambda st=st: wo_st(st)) for st in range(bl * 4, bl * 4 + 4)]

        attn = attn_pair_pipe if PIPE else attn_pair

        if FILL:
            for f in proj_pieces(0):
                f()
            for qb in range(4):
                fillers = []
                if qb + 1 < 4:
                    fillers += proj_pieces(qb + 1)
                if qb >= 1:
                    fillers += wo_pieces(qb - 1)
                fill_iter = iter(fillers)
                attn(qb, 0, fill_iter)
                attn(qb, 1, fill_iter)
                for f in fill_iter:
                    f()
            for f in wo_pieces(3):
                f()
        else:
            for nb in range(4):
                for f in proj_pieces(nb):
                    f()
                if nb >= 1:
                    for f in wo_pieces(nb - 1):
                        f()
                attn(nb, 0)
                attn(nb, 1)
            for f in wo_pieces(3):
                f()

    nc.compile()
    return nc


def prepare_in_maps(x, wq, bq, wk, bk, wv, bv, wo):
    x = np.asarray(x, dtype=np.float32)
    xb = [np.ascontiguousarray(x[b].T).astype(BF16) for b in range(B)]
    wqb = np.asarray(wq, dtype=np.float32).astype(BF16)
    wkb = np.asarray(wk, dtype=np.float32).astype(BF16)
    wvb = np.asarray(wv, dtype=np.float32).astype(BF16)
    wob = np.asarray(wo, dtype=np.float32).astype(BF16)
    bq = np.asarray(bq, dtype=np.float32)
    bk = np.asarray(bk, dtype=np.float32)
    bv = np.asarray(bv, dtype=np.float32)

    in_maps = []
    for c in range(N_CORES):
        b, g = c // 4, c % 4
        sq = slice(g * DG, (g + 1) * DG)
        sk = slice(g * HD, (g + 1) * HD)
        in_maps.append({
            "xtd": xb[b],
            "wqd": np.ascontiguousarray(wqb[:, sq]),
            "wkvd": np.ascontiguousarray(
                np.concatenate([wkb[:, sk], wvb[:, sk]], axis=1)),
            "wod": np.ascontiguousarray(wob[sq, :]),
            "bqd": np.ascontiguousarray((bq[sq] * 0.125).reshape(DG, 1)),
            "bkvd": np.ascontiguousarray(
                np.concatenate([bk[sk], bv[sk]]).reshape(2 * HD, 1)),
        })
    return in_maps


def kernel(x, mask, wq, bq, wk, bk, wv, bv, wo, bo):
    bo = np.asarray(bo, dtype=np.float32)
    in_maps = prepare_in_maps(x, wq, bq, wk, bk, wv, bv, wo)
    results = _run(in_maps)

    out = np.empty((B, S, D), dtype=np.float32)
    for b in range(B):
        acc = results[b * 4 + 0]["out_p"].astype(np.float64)
        for g in range(1, 4):
            acc += results[b * 4 + g]["out_p"]
        out[b] = (acc + bo).astype(np.float32)
    return out


def _get_runner():
    """Build (once) a jitted shard_map callable executing the compiled
    kernel on 8 cores. Adapted from concourse.bass2jax.run_bass_via_pjrt,
    minus output-buffer donation so the callable is re-invokable for
    timing."""
    if "runner" in _CACHE:
        return _CACHE["runner"]
    import jax
    from jax.experimental.shard_map import shard_map
    from jax.sharding import Mesh, PartitionSpec
    from concourse import bass2jax
    from concourse.bass2jax import _bass_exec_p, install_neuronx_cc_hook

    install_neuronx_cc_hook()
    nc = build_nc()
    partition_name = (
        nc.partition_id_tensor.name if nc.partition_id_tensor else None
    )

    in_names, out_names, out_avals, zero_outs = [], [], [], []
    for alloc in nc.m.functions[0].allocations:
        if not isinstance(alloc, mybir.MemoryLocationSet):
            continue
        name = alloc.memorylocations[0].name
        if alloc.kind == "ExternalInput":
            if name != partition_name:
                in_names.append(name)
        elif alloc.kind == "ExternalOutput":
            out_names.append(name)
            shape = tuple(alloc.tensor_shape)
            dtype = mybir.dt.np(alloc.dtype)
            out_avals.append(jax.core.ShapedArray(shape, dtype))
            zero_outs.append(np.zeros(shape, dtype))
    n_params = len(in_names)
    all_names = in_names + out_names
    if partition_name is not None:
        all_names = all_names + [partition_name]

    def _body(*args):
        operands = list(args)
        if partition_name is not None:
            operands.append(bass2jax.partition_id_tensor())
        outs = _bass_exec_p.bind(
            *operands,
            out_avals=tuple(out_avals),
            in_names=tuple(all_names),
            out_names=tuple(out_names),
            lowering_input_output_aliases=(),
            sim_require_finite=True,
            sim_require_nnan=True,
            nc=nc,
        )
        return tuple(outs)

    devices = jax.devices()[:N_CORES]
    mesh = Mesh(np.asarray(devices), ("core",))
    n_all = n_params + len(out_names)
    sharded = jax.jit(
        shard_map(
            _body,
            mesh=mesh,
            in_specs=(PartitionSpec("core"),) * n_all,
            out_specs=(PartitionSpec("core"),) * len(out_names),
            check_rep=False,
        ),
        keep_unused=True,
    )
    runner = {
        "sharded": sharded,
        "in_names": in_names,
        "out_names": out_names,
        "out_avals": out_avals,
        "zero_outs": zero_outs,
        "mesh": mesh,
        "nc": nc,
    }
    _CACHE["runner"] = runner
    return runner


def _run(in_maps):
    r = _get_runner()
    concat_in = [
        np.concatenate([np.asarray(in_maps[c][n]) for c in range(N_CORES)], axis=0)
        for n in r["in_names"]
    ]
    concat_zeros = [
        np.zeros((N_CORES * z.shape[0], *z.shape[1:]), z.dtype)
        for z in r["zero_outs"]
    ]
    out_arrs = r["sharded"](*concat_in, *concat_zeros)
    _CACHE["last_args"] = (concat_in, concat_zeros)
    return [
        {
            n: np.asarray(out_arrs[i]).reshape(
                N_CORES, *r["out_avals"][i].shape
            )[c]
            for i, n in enumerate(r["out_names"])
        }
        for c in range(N_CORES)
    ]


def bench(iters=10):
    """Re-execute the last-run kernel with device-resident inputs and
    return per-call wall times (s). Outputs stay on device."""
    import time as _time
    import jax
    from jax.sharding import NamedSharding, PartitionSpec

    r = _CACHE["runner"]
    concat_in, concat_zeros = _CACHE["last_args"]
    sh = NamedSharding(r["mesh"], PartitionSpec("core"))
    dev_args = [jax.device_put(a, sh) for a in (*concat_in, *concat_zeros)]
    for a in dev_args:
        a.block_until_ready()
    times = []
    for _ in range(iters):
        t0 = _time.perf_counter()
        outs = r["sharded"](*dev_args)
        for o in outs:
            o.block_until_ready()
        times.append(_time.perf_counter() - t0)
    return times
